# revision 1
# baseline (speedup 1.0000x reference)
"""GatedAttention Trainium2 kernel (8 NeuronCores, tensor-parallel over (batch, head-group)).

Sharding: core c handles batch b=c//4 and heads 4*(c%4)..4*(c%4)+3.
Each core computes qkv/gate projections for its heads from x[b], per-head
QK-RMS-norm + softmax attention + sigmoid gating, and a row-split o_proj
partial [S, D]. Host sums the 4 partials per batch and adds the residual.

Math notes:
- prenorm RMS scale r[s] cancels inside q/k RMS-norm, so q/k use raw x;
  r is only applied to the v/gate path (host-precomputed, fused into the
  v/gate PSUM->SBUF copyback scale).
- prenorm_w is folded into the projection weights on host.
- softmax runs without max-subtraction (scores are QK-normalized, |s|<~6).
- attention 1/sqrt(64) scale and q-norm are folded into rq = 1/sqrt(sumsq+64*eps).
- v carries an extra all-ones column so attn@v also yields the softmax sums.
- matmuls run in bf16 (fp32 PSUM accumulation); softmax/norm math in fp32.
"""

import json

import numpy as np
import ml_dtypes

import concourse.bass as bass
import concourse.bass_utils as bass_utils
import concourse.bass2jax as bass2jax
import concourse.mybir as mybir
import concourse.tile as tile
from concourse.tile import TileContext
from concourse.masks import make_identity
from concourse.vector_clock import ScopedClock, VectorClock

F32 = mybir.dt.float32
BF16 = mybir.dt.bfloat16
AF = mybir.ActivationFunctionType
BF = ml_dtypes.bfloat16

B, S, D = 2, 2048, 1024
NH_TOT, HD = 16, 64
NH = 4            # heads per core
EPS = 1e-5
P = 128
ST = S // P       # 16 s-tiles
KT = D // P       # 8 d-tiles
NCH = S // 512    # 4 sq chunks

# ----------------------------------------------------------------------------
# compat patches: this walrus build accepts only ONE sync-wait per instruction
# ----------------------------------------------------------------------------

def _patched_drain_and_barrier(self, tick_clock, wait_clock):
    nc = self.nc
    gc = tick_clock.global_clock
    n = len(gc)
    for p in range(n):
        t = gc[p]
        if t <= 0:
            continue
        vec = VectorClock([0] * n)
        vec.require_at_least(p, t)
        nop = nc.sync.nop(nofuse=True, hint=f"drain_wait_p{p}")
        wait_clock.add_sem_waits(nop.ins, ScopedClock({None: vec}))
    nc.sync.drain(fusable=False)
    nc.all_engine_barrier()
    assert self.sems is not None
    popped = nc._tile_sem_poison_stack.pop()
    assert popped is self._sem_poison
    nc.clear_and_free_semaphores(list(self.sems.allocated().values()))
    nc.all_engine_barrier()


def _split_multi_waits(bir_json: bytes) -> bytes:
    bj = json.loads(bir_json)
    n_split = 0
    for fn in bj.get("functions", []):
        for blk in fn.get("blocks", []):
            out = []
            for inst in blk.get("instructions", []):
                si = inst.get("sync_info")
                waits = si.get("on_wait", []) if si else []
                if len(waits) > 1:
                    for i, w in enumerate(waits[:-1]):
                        out.append({
                            "debug": inst.get("debug"),
                            "engine": inst["engine"],
                            "ins": [], "outs": [],
                            "name": f"{inst['name']}_sw{i}",
                            "opcode": "NoOp",
                            "sync_info": {"on_update": [], "on_wait": [w]},
                            "text_hint": "split_wait",
                        })
                        n_split += 1
                    si["on_wait"] = [waits[-1]]
                out.append(inst)
            blk["instructions"] = out
    if n_split:
        return json.dumps(bj).encode()
    return bir_json


_ORIG_COMPILE = bass_utils.compile_bir_kernel


def _patched_compile_bir_kernel(bir_json, tmpdir, neff_name="file.neff"):
    return _ORIG_COMPILE(_split_multi_waits(bir_json), tmpdir, neff_name)


def _apply_compat():
    tile.TileContext._drain_and_barrier = _patched_drain_and_barrier
    bass_utils.compile_bir_kernel = _patched_compile_bir_kernel
    bass2jax.compile_bir_kernel = _patched_compile_bir_kernel


_apply_compat()

# ----------------------------------------------------------------------------
# device program (SPMD: identical program, per-core data)
# ----------------------------------------------------------------------------

_NC_CACHE = None


def _build_program():
    nc = bass.Bass()
    xt = nc.declare_dram_parameter("xt", [P, KT, S], BF16, isOutput=False)
    wqk = nc.declare_dram_parameter("wqk", [P, KT, 512], BF16, isOutput=False)
    wvg = nc.declare_dram_parameter("wvg", [P, KT, 260], BF16, isOutput=False)
    wo = nc.declare_dram_parameter("wo", [64, NH, D], BF16, isOutput=False)
    rv = nc.declare_dram_parameter("rv", [P, ST], F32, isOutput=False)
    qn = nc.declare_dram_parameter("qn", [P, 1], F32, isOutput=False)
    kn = nc.declare_dram_parameter("kn", [P, 1], F32, isOutput=False)
    ind = nc.declare_dram_parameter("ind", [P, 2], BF16, isOutput=False)
    ind2 = nc.declare_dram_parameter("ind2", [2, P], F32, isOutput=False)
    one64 = nc.declare_dram_parameter("one64", [1, 64], F32, isOutput=False)
    outp = nc.declare_dram_parameter("out_p", [S, D], F32, isOutput=True)

    with TileContext(nc) as tc:
        with tc.tile_pool(name="big", bufs=1) as big, \
             tc.tile_pool(name="work", bufs=3) as work, \
             tc.tile_pool(name="wbig", bufs=1) as wbig, \
             tc.tile_pool(name="pacc", bufs=3, space="PSUM") as pacc, \
             tc.tile_pool(name="pstr", bufs=5, space="PSUM") as pstr:

            # ---- resident inputs
            xts = big.tile([P, KT, S], BF16)
            nc.sync.dma_start(out=xts[:], in_=xt[:, :, :])
            wqks = big.tile([P, KT, 512], BF16)
            nc.sync.dma_start(out=wqks[:], in_=wqk[:, :, :])
            wvgs = big.tile([P, KT, 260], BF16)
            nc.sync.dma_start(out=wvgs[:], in_=wvg[:, :, :])
            wos = big.tile([64, NH, D], BF16)
            nc.sync.dma_start(out=wos[:], in_=wo[:, :, :])
            rvs = big.tile([P, ST], F32)
            nc.sync.dma_start(out=rvs[:], in_=rv[:, :])
            qns = big.tile([P, 1], F32)
            nc.sync.dma_start(out=qns[:], in_=qn[:, :])
            kns = big.tile([P, 1], F32)
            nc.sync.dma_start(out=kns[:], in_=kn[:, :])
            inds = big.tile([P, 2], BF16)
            nc.sync.dma_start(out=inds[:], in_=ind[:, :])
            ind2s = big.tile([2, P], F32)
            nc.sync.dma_start(out=ind2s[:], in_=ind2[:, :])
            one64s = big.tile([1, 64], F32)
            nc.sync.dma_start(out=one64s[:], in_=one64[:, :])

            ident = big.tile([P, P], F32)
            make_identity(nc, ident[:])
            epsb = big.tile([P, 1], F32)
            nc.vector.memset(epsb[:], EPS)
            eps64 = big.tile([P, 1], F32)
            nc.vector.memset(eps64[:], HD * EPS)

            # ---- resident intermediates
            qkT = big.tile([P, 4, S], BF16)        # mt0,1=q(pair0,1) mt2,3=k
            vbuf = big.tile([P, ST, NH, 65], BF16)  # [sk%128, skt, head, hd+ones]
            nc.vector.memset(vbuf[:], 1.0)
            gnat = big.tile([P, ST, NH], F32)       # sigmoid(r*gate) [s-part]
            gtr = big.tile([1, NH, S], F32)         # gate rows at partition 0
            rkb = big.tile([P, ST, NH], F32)        # rsqrt per (sk, head)
            obuf = big.tile([64, NH, S], BF16)      # attn outT per head
            crb = big.tile([1, NH, S], F32)         # gate*recip(sums) rows

            # ---- phase C: qk projection (+ raw sumsq)
            for mt in range(4):
                ssum = (wbig.tile([2, S], F32, tag="ssum", name="ssum")
                        if mt < 2 else None)
                for ch in range(NCH):
                    pq = pacc.tile([P, 512], F32, tag="acc")
                    for kt in range(KT):
                        nc.tensor.matmul(
                            pq[:], wqks[:, kt, 128 * mt:128 * mt + 128],
                            xts[:, kt, 512 * ch:512 * ch + 512],
                            start=(kt == 0), stop=(kt == KT - 1))
                    sc = qns if mt < 2 else kns
                    nc.scalar.activation(
                        qkT[:, mt, 512 * ch:512 * ch + 512], pq[:], AF.Copy,
                        scale=sc[:])
                    # squares (raw, pre-norm-weight) for sumsq
                    sq = work.tile([P, 512], BF16, tag="sq")
                    nc.scalar.activation(sq[:], pq[:], AF.Square)
                    if mt < 2:  # q: row-layout sums [2, 512]
                        pr = pstr.tile([2, 512], F32, tag="str")
                        nc.tensor.matmul(pr[:], inds[:], sq[:],
                                         start=True, stop=True)
                        nc.vector.tensor_copy(
                            out=ssum[:, 512 * ch:512 * ch + 512], in_=pr[:])
                    else:  # k: column-layout sums [128, 2] per 128-slice
                        for sl in range(4):
                            pk = pstr.tile([P, 2], F32, tag="str")
                            nc.tensor.matmul(
                                pk[:], sq[:, 128 * sl:128 * sl + 128], inds[:],
                                start=True, stop=True)
                            tmp = work.tile([P, 2], F32, tag="rk_t")
                            nc.scalar.activation(tmp[:], pk[:], AF.Sqrt,
                                                 bias=epsb[:], scale=1.0 / HD)
                            skt = ch * 4 + sl
                            j0 = (mt - 2) * 2
                            nc.vector.reciprocal(
                                rkb[:, skt, j0:j0 + 2], tmp[:])

                # rq + scale q rows for this mt
                if mt < 2:
                    tmp = wbig.tile([2, S], F32, tag="rq_t")
                    nc.scalar.activation(tmp[:], ssum[:], AF.Sqrt,
                                         bias=eps64[0:2, :], scale=1.0)
                    rqb = wbig.tile([2, S], F32, tag="rqb")
                    nc.vector.reciprocal(rqb[:], tmp[:])
                    for ch in range(NCH):
                        pbc = pstr.tile([P, 512], F32, tag="str")
                        nc.tensor.matmul(pbc[:], ind2s[:],
                                         rqb[:, 512 * ch:512 * ch + 512],
                                         start=True, stop=True)
                        nc.vector.tensor_tensor(
                            qkT[:, mt, 512 * ch:512 * ch + 512],
                            qkT[:, mt, 512 * ch:512 * ch + 512], pbc[:],
                            mybir.AluOpType.mult)

            # ---- phase E: v + gate projection
            for t in range(ST):
                pv = pacc.tile([P, 512], F32, tag="acc")
                for kt in range(KT):
                    nc.tensor.matmul(pv[:, 0:260], xts[:, kt, 128 * t:128 * t + 128],
                                     wvgs[:, kt, :],
                                     start=(kt == 0), stop=(kt == KT - 1))
                nc.scalar.activation(vbuf[:, t, :, 0:64], pv[:, 0:256],
                                     AF.Copy, scale=rvs[:, t:t + 1])
                nc.scalar.activation(gnat[:, t, :], pv[:, 256:260],
                                     AF.Sigmoid, scale=rvs[:, t:t + 1])

            # ---- phase F: gate transpose -> per-head rows at partition 0
            for t in range(ST):
                for j in range(NH):
                    pg = pstr.tile([P, 512], F32, tag="str")
                    nc.tensor.transpose(pg[0:1, 0:128], gnat[:, t, j:j + 1],
                                        ident[:])
                    nc.vector.tensor_copy(
                        out=gtr[0:1, j, 128 * t:128 * t + 128],
                        in_=pg[0:1, 0:128])

            # ---- phase G: attention per head
            for j in range(NH):
                a, hp = 64 * (j % 2), j // 2
                for ch in range(NCH):
                    po = pacc.tile([65, 512], F32, tag="acc")
                    for skt in range(ST):
                        ps = pstr.tile([P, 512], F32, tag="str")
                        nc.tensor.matmul(
                            ps[:],
                            qkT[a:a + 64, 2 + hp, 128 * skt:128 * skt + 128],
                            qkT[a:a + 64, hp, 512 * ch:512 * ch + 512],
                            start=True, stop=True)
                        ex = work.tile([P, 512], BF16, tag="ex")
                        nc.scalar.activation(ex[:], ps[:], AF.Exp,
                                             scale=rkb[:, skt, j:j + 1])
                        nc.tensor.matmul(po[:], vbuf[:, skt, j, 0:65], ex[:],
                                         start=(skt == 0), stop=(skt == ST - 1))
                    # finalize: combined = sigmoid(gate)/sums, bcast, apply
                    fin = work.tile([65, 512], F32, tag="fin")
                    nc.vector.reciprocal(fin[64:65, :], po[64:65, :])
                    fr = work.tile([1, 512], F32, tag="fr")
                    nc.sync.dma_start(out=fr[:], in_=fin[64:65, :])
                    nc.vector.tensor_tensor(
                        crb[0:1, j, 512 * ch:512 * ch + 512], fr[:],
                        gtr[0:1, j, 512 * ch:512 * ch + 512],
                        mybir.AluOpType.mult)
                    nc.vector.tensor_copy(
                        out=obuf[:, j, 512 * ch:512 * ch + 512], in_=po[0:64, :])

            # ---- batched normalization (keeps PE dense during attention)
            for j in range(NH):
                for ch in range(NCH):
                    pbc = pstr.tile([P, 512], F32, tag="str")
                    nc.tensor.matmul(
                        pbc[0:64, :], one64s[:],
                        crb[0:1, j, 512 * ch:512 * ch + 512],
                        start=True, stop=True)
                    nc.vector.tensor_tensor(
                        obuf[:, j, 512 * ch:512 * ch + 512],
                        obuf[:, j, 512 * ch:512 * ch + 512], pbc[0:64, :],
                        mybir.AluOpType.mult)

            # ---- phase H: o_proj partial [S, D]
            for t in range(ST):
                for nh in range(2):
                    pp = pacc.tile([P, 512], F32, tag="acc")
                    for j in range(NH):
                        nc.tensor.matmul(
                            pp[:], obuf[:, j, 128 * t:128 * t + 128],
                            wos[:, j, 512 * nh:512 * nh + 512],
                            start=(j == 0), stop=(j == NH - 1))
                    ot = work.tile([P, 512], F32, tag="ot")
                    nc.vector.tensor_copy(out=ot[:], in_=pp[:])
                    nc.sync.dma_start(
                        out=outp[128 * t:128 * t + 128, 512 * nh:512 * nh + 512],
                        in_=ot[:])
    return nc


def _get_program():
    global _NC_CACHE
    if _NC_CACHE is None:
        _NC_CACHE = _build_program()
    return _NC_CACHE


# ----------------------------------------------------------------------------
# host wrapper
# ----------------------------------------------------------------------------

def _prep_inputs(x, prenorm_w, qkv_w, gate_w, o_w, q_norm_w, k_norm_w):
    x = np.asarray(x, np.float32)
    pw = np.asarray(prenorm_w, np.float32)
    qkv_w = np.asarray(qkv_w, np.float32)
    gate_w = np.asarray(gate_w, np.float32)
    o_w = np.asarray(o_w, np.float32)
    qw = qkv_w[0:D] * pw[None, :]
    kw = qkv_w[D:2 * D] * pw[None, :]
    vw = qkv_w[2 * D:3 * D] * pw[None, :]
    gw = gate_w * pw[None, :]

    r = 1.0 / np.sqrt(np.mean(x * x, axis=-1) + EPS)      # [B, S]
    ind = np.zeros((P, 2), BF)
    ind[0:64, 0] = 1
    ind[64:128, 1] = 1
    ind2 = np.zeros((2, P), np.float32)
    ind2[0, 0:64] = 1
    ind2[1, 64:128] = 1
    one64 = np.ones((1, 64), np.float32)
    qn = np.tile(np.asarray(q_norm_w, np.float32), 2)[:, None]
    kn = np.tile(np.asarray(k_norm_w, np.float32), 2)[:, None]

    in_maps = []
    for c in range(8):
        b, hg = c // 4, c % 4
        hsl = slice(256 * hg, 256 * hg + 256)
        xtc = np.ascontiguousarray(
            x[b].T.reshape(KT, P, S).transpose(1, 0, 2)).astype(BF)
        wqk = np.concatenate([qw[hsl], kw[hsl]], 0).T  # [1024, 512]
        wqkc = np.ascontiguousarray(
            wqk.reshape(KT, P, 512).transpose(1, 0, 2)).astype(BF)
        wvg = np.concatenate([vw[hsl], gw[4 * hg:4 * hg + 4]], 0).T  # [1024,260]
        wvgc = np.ascontiguousarray(
            wvg.reshape(KT, P, 260).transpose(1, 0, 2)).astype(BF)
        wo = o_w[:, hsl].T.reshape(NH, 64, D).transpose(1, 0, 2)  # [64, NH, D]
        woc = np.ascontiguousarray(wo).astype(BF)
        rvc = np.ascontiguousarray(r[b].reshape(ST, P).T).astype(np.float32)
        in_maps.append({
            "xt": xtc, "wqk": wqkc, "wvg": wvgc, "wo": woc,
            "rv": rvc, "qn": qn.astype(np.float32),
            "kn": kn.astype(np.float32), "ind": ind, "ind2": ind2,
            "one64": one64,
        })
    return in_maps


_RUNNER = None


def _get_runner():
    """Build the sharded PJRT executable ONCE and reuse it across calls
    (run_bass_kernel_spmd re-traces/re-compiles on every invocation)."""
    global _RUNNER
    if _RUNNER is not None:
        return _RUNNER
    import jax
    import concourse.mybir as _mybir
    from concourse.bass2jax import (_bass_exec_p, partition_id_tensor,
                                    install_neuronx_cc_hook, Mesh,
                                    PartitionSpec, shard_map)
    install_neuronx_cc_hook()
    nc = _get_program()
    n_cores = 8
    partition_name = (nc.partition_id_tensor.name
                      if nc.partition_id_tensor else None)
    in_names, out_names, out_avals, zero_outs = [], [], [], []
    for alloc in nc.m.functions[0].allocations:
        if not isinstance(alloc, _mybir.MemoryLocationSet):
            continue
        name = alloc.memorylocations[0].name
        if alloc.kind == "ExternalInput":
            if name != partition_name:
                in_names.append(name)
        elif alloc.kind == "ExternalOutput":
            shape = tuple(alloc.tensor_shape)
            dtype = _mybir.dt.np(alloc.dtype)
            out_names.append(name)
            out_avals.append(jax.core.ShapedArray(shape, dtype))
            zero_outs.append(np.zeros(shape, dtype))
    n_params = len(in_names)
    n_outs = len(out_avals)
    all_in = list(in_names) + list(out_names)
    if partition_name is not None:
        all_in.append(partition_name)
    donate = tuple(range(n_params, n_params + n_outs))

    def _body(*args):
        operands = list(args)
        if partition_name is not None:
            operands.append(partition_id_tensor())
        return tuple(_bass_exec_p.bind(
            *operands, out_avals=tuple(out_avals), in_names=tuple(all_in),
            out_names=tuple(out_names), lowering_input_output_aliases=(),
            sim_require_finite=True, sim_require_nnan=True, nc=nc))

    devices = jax.devices()[:n_cores]
    mesh = Mesh(np.asarray(devices), ("core",))
    sharded = jax.jit(
        shard_map(_body, mesh=mesh,
                  in_specs=(PartitionSpec("core"),) * (n_params + n_outs),
                  out_specs=(PartitionSpec("core"),) * n_outs,
                  check_rep=False),
        donate_argnums=donate, keep_unused=True)
    _RUNNER = (sharded, in_names, out_names, out_avals, zero_outs, n_cores)
    return _RUNNER


def kernel(x, prenorm_w, qkv_w, gate_w, o_w, q_norm_w, k_norm_w):
    sharded, in_names, out_names, out_avals, zero_outs, n_cores = _get_runner()
    in_maps = _prep_inputs(x, prenorm_w, qkv_w, gate_w, o_w,
                           q_norm_w, k_norm_w)
    concat_in = [np.concatenate([in_maps[c][nm] for c in range(n_cores)], 0)
                 for nm in in_names]
    concat_zeros = [np.zeros((n_cores * z.shape[0], *z.shape[1:]), z.dtype)
                    for z in zero_outs]
    out_arrs = sharded(*concat_in, *concat_zeros)
    oi = out_names.index("out_p")
    op = np.asarray(out_arrs[oi]).reshape(n_cores, *out_avals[oi].shape)
    outs = [op[c] for c in range(n_cores)]
    x = np.asarray(x, np.float32)
    y0 = x[0] + outs[0] + outs[1] + outs[2] + outs[3]
    y1 = x[1] + outs[4] + outs[5] + outs[6] + outs[7]
    return np.stack([y0, y1]).astype(np.float32)



# revision 44
# speedup vs baseline: 1.4670x; 1.4670x over previous
"""GatedAttention Trainium2 kernel (8 NeuronCores, tensor-parallel over (batch, head-group)).

Sharding: core c handles batch b=c//4 and heads 4*(c%4)..4*(c%4)+3.
Each core computes qkv/gate projections for its heads from x[b], per-head
QK-RMS-norm + softmax attention + sigmoid gating, and a row-split o_proj
partial [S, D] (bf16). Host sums the 4 partials per batch + residual.

Key structure:
- All big matmuls run in fp8e4m3 with DoubleRow (contraction pairs packed
  into the free dim of both operands: [Ki, 2, M] x [Ki, 2, N]). Weights are
  pre-scaled x32 on host so fp8 sees ~unit-variance values; descales fold
  into copyback scales.
- attnV emits [sq, hd] tiles (out free = 65) instead of [hd, sq]: softmax
  sums + sigmoid gate become per-partition scalars. A per-sq-tile PE
  transpose rebuilds the [hd, sq] layout o_proj needs, writing into the
  just-drained po PSUM slot.
- exp is the elementwise wall (16.8M elems/core): batched to N=1024 per
  instruction and split between ACT (native Exp) and a custom single-pass
  8-stage DVE op computing exp(t) ~= (1 + t/64)^64.
- softmax runs without max-subtraction; a uniform -3*ln2 bias keeps exp
  outputs under the fp8e4m3 max (e^(8-2.079)=372<448; |s|<=8 by
  Cauchy-Schwarz after QK RMS norm). The bias cancels in normalization.
- v carries an all-ones column so attnV also yields the softmax sums.
"""

import json
import math

import numpy as np
import ml_dtypes

import concourse.bass as bass
import concourse.bass_utils as bass_utils
import concourse.bass2jax as bass2jax
import concourse.mybir as mybir
import concourse.tile as tile
import concourse.dve_ops as dve_ops
from concourse.dve_ops import DveOp
from concourse.dve_spec import Spec, Src0, C0, C1, sq as dve_sq
from concourse.tile import TileContext
from concourse.masks import make_identity
from concourse.vector_clock import ScopedClock, VectorClock

F32 = mybir.dt.float32
BF16 = mybir.dt.bfloat16
F8 = mybir.dt.float8e4
AF = mybir.ActivationFunctionType
ALU = mybir.AluOpType
DR = mybir.MatmulPerfMode.DoubleRow
BF = ml_dtypes.bfloat16
F8NP = mybir.dt.np(mybir.dt.float8e4)

B, S, D = 2, 2048, 1024
NH_TOT, HD = 16, 64
NH = 4            # heads per core
EPS = 1e-5
P = 128
ST = S // P       # 16 s-tiles
KTP = 4           # d-dim pair-tiles (4 x (2x128))
EXPB = 3 * math.log(2.0)          # uniform score bias (cancels in softmax)
C1EXP = 1.0 - EXPB / 64.0         # dve exp: a = s*rk/64 + C1EXP

# engine split knobs (tuned against sim engine-busy readout)
# NOTE: this container's walrus cannot codegen custom-DVE ops ("ISA wrong
# length" even for the production RECIPROCAL_APPROX_FAST), so exp runs
# entirely on ACT and everything else moves to DVE.
EXP_ACT_OF_32 = 32   # of the 32 exp half-tiles per head, this many on ACT

# ----------------------------------------------------------------------------
# custom DVE op: exp(t) ~= (1 + t/64)^64, one pass, 8 uop stages
# ----------------------------------------------------------------------------


def _ref_exp64(in0, in1, s0, s1, imm2):
    a = in0.astype(np.float32) * np.asarray(s0, np.float32) + np.float32(s1)
    for _ in range(6):
        a = a * a
    return a


EXP64_ANT = DveOp(
    "EXP64_ANT",
    Spec(
        body=dve_sq(dve_sq(dve_sq(dve_sq(dve_sq(dve_sq(Src0 * C0 + C1)))))),
        reference=_ref_exp64,
    ),
    subdim=False,
    uops_sha={"v3": "8299cc4e9a89acf1", "v4": "df7b3d1456faeb1a"},
)


def _register_exp_op():
    if EXP64_ANT.name in dve_ops.CUSTOM_DVE_SPECS:
        return
    row = max(dve_ops._SUB_OPCODE_FOR_NAME.values()) + 1
    assert row < 0x20
    dve_ops.OPS.append(EXP64_ANT)
    dve_ops.CUSTOM_DVE_SPECS[EXP64_ANT.name] = EXP64_ANT.spec
    dve_ops._SUB_OPCODE_FOR_NAME[EXP64_ANT.name] = row


_register_exp_op()

# ----------------------------------------------------------------------------
# compat patches: this walrus build accepts only ONE sync-wait per instruction
# ----------------------------------------------------------------------------

def _patched_drain_and_barrier(self, tick_clock, wait_clock):
    nc = self.nc
    gc = tick_clock.global_clock
    n = len(gc)
    for p in range(n):
        t = gc[p]
        if t <= 0:
            continue
        vec = VectorClock([0] * n)
        vec.require_at_least(p, t)
        nop = nc.sync.nop(nofuse=True, hint=f"drain_wait_p{p}")
        wait_clock.add_sem_waits(nop.ins, ScopedClock({None: vec}))
    nc.sync.drain(fusable=False)
    nc.all_engine_barrier()
    assert self.sems is not None
    popped = nc._tile_sem_poison_stack.pop()
    assert popped is self._sem_poison
    nc.clear_and_free_semaphores(list(self.sems.allocated().values()))
    nc.all_engine_barrier()


def _split_multi_waits(bir_json: bytes) -> bytes:
    bj = json.loads(bir_json)
    n_split = 0
    for fn in bj.get("functions", []):
        for blk in fn.get("blocks", []):
            out = []
            for inst in blk.get("instructions", []):
                si = inst.get("sync_info")
                waits = si.get("on_wait", []) if si else []
                if len(waits) > 1:
                    for i, w in enumerate(waits[:-1]):
                        out.append({
                            "debug": inst.get("debug"),
                            "engine": inst["engine"],
                            "ins": [], "outs": [],
                            "name": f"{inst['name']}_sw{i}",
                            "opcode": "NoOp",
                            "sync_info": {"on_update": [], "on_wait": [w]},
                            "text_hint": "split_wait",
                        })
                        n_split += 1
                    si["on_wait"] = [waits[-1]]
                out.append(inst)
            blk["instructions"] = out
    if n_split:
        return json.dumps(bj).encode()
    return bir_json


_ORIG_COMPILE = bass_utils.compile_bir_kernel


def _patched_compile_bir_kernel(bir_json, tmpdir, neff_name="file.neff"):
    return _ORIG_COMPILE(_split_multi_waits(bir_json), tmpdir, neff_name)


def _apply_compat():
    tile.TileContext._drain_and_barrier = _patched_drain_and_barrier
    bass_utils.compile_bir_kernel = _patched_compile_bir_kernel
    bass2jax.compile_bir_kernel = _patched_compile_bir_kernel


_apply_compat()

# ----------------------------------------------------------------------------
# device program (SPMD: identical program, per-core data)
# ----------------------------------------------------------------------------

_NC_CACHE = None


def _build_program():
    nc = bass.Bass()
    xt8 = nc.declare_dram_parameter("xt8", [P, KTP, 2, S], F8, isOutput=False)
    wqk8 = nc.declare_dram_parameter("wqk8", [P, KTP, 2, 512], F8, isOutput=False)
    wvg8 = nc.declare_dram_parameter("wvg8", [P, KTP, 2, 272], F8, isOutput=False)
    wo8 = nc.declare_dram_parameter("wo8", [64, NH, D], F8, isOutput=False)
    rv = nc.declare_dram_parameter("rv", [P, ST], F32, isOutput=False)
    rv4 = nc.declare_dram_parameter("rv4", [P, 64], F32, isOutput=False)
    qn = nc.declare_dram_parameter("qn", [P, 1], F32, isOutput=False)
    kn = nc.declare_dram_parameter("kn", [P, 1], F32, isOutput=False)
    ind = nc.declare_dram_parameter("ind", [P, 2], BF16, isOutput=False)
    ind2 = nc.declare_dram_parameter("ind2", [2, P], BF16, isOutput=False)
    outp = nc.declare_dram_parameter("out_p", [S, D], BF16, isOutput=True)

    with TileContext(nc) as tc:
        with tc.tile_pool(name="big", bufs=1) as big, \
             tc.tile_pool(name="work", bufs=4) as work, \
             tc.tile_pool(name="wex", bufs=9) as wex, \
             tc.tile_pool(name="wsb", bufs=4) as wsb, \
             tc.tile_pool(name="wot", bufs=4) as wot, \
             tc.tile_pool(name="psc", bufs=2, space="PSUM") as psc, \
             tc.tile_pool(name="ppo", bufs=1, space="PSUM") as ppo:

            # ---- resident inputs (weights first; x chunked by s-range so
            # the first projection iterations start early; wo8 needed last)
            wqks = big.tile([P, KTP, 2, 512], F8)
            nc.sync.dma_start(out=wqks[:], in_=wqk8[:, :, :, :])
            xts = big.tile([P, KTP, 2, S], F8)
            for xc in range(4):
                nc.sync.dma_start(
                    out=xts[:, :, :, 512 * xc:512 * xc + 512],
                    in_=xt8[:, :, :, 512 * xc:512 * xc + 512])
            wvgs = big.tile([P, KTP, 2, 272], F8)
            nc.sync.dma_start(out=wvgs[:], in_=wvg8[:, :, :, :])
            wos = big.tile([64, NH, D], F8)
            nc.gpsimd.dma_start(out=wos[:], in_=wo8[:, :, :])
            rvs = big.tile([P, ST], F32)
            nc.sync.dma_start(out=rvs[:], in_=rv[:, :])
            rv4s = big.tile([P, 64], F32)
            nc.sync.dma_start(out=rv4s[:], in_=rv4[:, :])
            qns = big.tile([P, 1], F32)
            nc.sync.dma_start(out=qns[:], in_=qn[:, :])
            kns = big.tile([P, 1], F32)
            nc.sync.dma_start(out=kns[:], in_=kn[:, :])
            inds = big.tile([P, 2], BF16)
            nc.sync.dma_start(out=inds[:], in_=ind[:, :])
            ind2s = big.tile([2, P], BF16)
            nc.sync.dma_start(out=ind2s[:], in_=ind2[:, :])

            ident = big.tile([P, P], F32)
            make_identity(nc, ident[:])
            epsb = big.tile([P, 1], F32)
            nc.vector.memset(epsb[:], EPS)
            eps64 = big.tile([P, 1], F32)
            nc.vector.memset(eps64[:], HD * EPS)
            expbb = big.tile([P, 1], F32)
            nc.vector.memset(expbb[:], -EXPB)

            # ---- resident intermediates
            qkT = big.tile([P, 4, S], BF16)          # mt0,1=q(pair0,1) mt2,3=k
            vbuf = big.tile([P, ST // 2, 2, NH, 68], F8)  # [sk%128,sktp,e,j,hd+1]
            nc.vector.memset(vbuf[:], 1.0)
            obuf = big.tile([64, NH, S], F8)         # gated attn outT per head
            gnat = big.tile([P, 16, 4], F32)         # sigmoid(r*gate) [t, j]
            rkb = big.tile([P, 64], F32)             # 1/rms(k), col=4skt+j
            rkb64 = big.tile([P, 64], F32)           # rkb/64 (dve exp scale)
            ssum = big.tile([2, 2, S], F32)          # q sumsq rows per mt
            rqb = big.tile([2, 2, S], BF16)          # q scale rows per mt
            tmpq = big.tile([2, 2, S], F32)
            tmpk = big.tile([P, 64], F32)
            graw = big.tile([P, 64], F32)
            rs = big.tile([P, 16], F32)              # recip softmax sums
            crb = big.tile([P, 16], F32)             # 32*gate*rs per sq-tile

            # po layout: [bank(4), slot(4), col(128)]; slice q -> [q//4, q%4]
            # pre-attention scratch carved from the same banks:
            #   gate psum = pot[:, 0, 0, 0:64]; k-sumsq psum = pot[:, 0, 1, 0:64]
            pot = ppo.tile([P, 4, 512], F32, tag="po")

            # k-norm column layout: kcol(skt, j) = 32*(j//2) + 2*skt + (j%2)
            # so each k head-pair's stats occupy one contiguous 32-col half.
            def emit_sums(mt, ch, sqt):
                if mt < 2:  # q: row-layout sums [2, 512]
                    prf = proj_psum()
                    nc.tensor.matmul(prf(512)[0:2], inds[:], sqt[:],
                                     start=True, stop=True)
                    nc.vector.tensor_copy(
                        out=ssum[:, mt, 512 * ch:512 * ch + 512],
                        in_=prf(512)[0:2])
                else:  # k: column sums
                    for sl in range(4):
                        skt = ch * 4 + sl
                        c0 = 128 + 32 * (mt - 2) + 2 * skt
                        nc.tensor.matmul(
                            pot[:, 0, c0:c0 + 2],
                            sqt[:, 128 * sl:128 * sl + 128], inds[:],
                            start=True, stop=True)

            # projection iteration; elementwise on ACT only where allowed
            # (anything on the ACT FIFO ahead of the exps delays attention).
            # Projection psum rotates over 4 slots: the 2-deep "sc" ring plus
            # pot banks 2/3, which attention doesn't touch until its q>=8
            # attnV writes (far later, ordered by the tile framework).
            pend = []
            pslot = [0]

            def proj_psum():
                s = pslot[0] = 2 + (pslot[0] + 1) % 2
                return lambda n, s=s: pot[:, s, 0:n]

            def emit_proj(mt, ch, act_ok):
                pqf = proj_psum()
                for ktp in range(KTP):
                    nc.tensor.matmul(
                        pqf(512),
                        wqks[:, ktp, :, 128 * mt:128 * mt + 128],
                        xts[:, ktp, :, 512 * ch:512 * ch + 512],
                        start=(ktp == 0), stop=(ktp == KTP - 1),
                        perf_mode=DR)
                sc = qns if mt < 2 else kns
                qsl = qkT[:, mt, 512 * ch:512 * ch + 512]
                if act_ok:
                    nc.scalar.activation(qsl, pqf(512), AF.Copy,
                                         scale=sc[:])
                else:
                    nc.vector.tensor_scalar(qsl, pqf(512), sc[:],
                                            None, op0=ALU.mult)
                # squares from the bf16 copy (sbuf 2-byte: fast DVE path)
                sqt = work.tile([P, 512], BF16, tag="sq")
                nc.vector.tensor_tensor(sqt[:], qsl, qsl, ALU.mult)
                pend.append((mt, ch, sqt))
                if len(pend) > 2:
                    emit_sums(*pend.pop(0))

            def emit_khalf(half):
                sl = slice(32 * half, 32 * half + 32)
                psl = slice(128 + 32 * half, 160 + 32 * half)
                nc.scalar.activation(tmpk[:, sl], pot[:, 0, psl],
                                     AF.Sqrt, bias=epsb[:], scale=1.0 / HD)
                nc.vector.reciprocal(rkb[:, sl], tmpk[:, sl])
                nc.vector.tensor_scalar(rkb64[:, sl], rkb[:, sl],
                                        1.0 / 64.0, None, op0=ALU.mult)

            def emit_qscale(mt, late=False):
                for chq in range(4):
                    if late:
                        pbct = psc.tile([P, 1024], F32, tag="sc", name="pbct")
                        pbc = lambda n: pbct[:, 0:n]
                    else:
                        pbc = proj_psum()
                    nc.tensor.matmul(
                        pbc(512), ind2s[:],
                        rqb[:, mt, 512 * chq:512 * chq + 512],
                        start=True, stop=True)
                    nc.vector.tensor_tensor(
                        qkT[:, mt, 512 * chq:512 * chq + 512],
                        qkT[:, mt, 512 * chq:512 * chq + 512],
                        pbc(512), ALU.mult)

            # ---- phase C part 1: mt0 (q heads 0/1) + mt2 (k heads 0/1)
            for it, (mt, ch) in enumerate(
                    [(m, c) for m in (0, 2) for c in range(4)]):
                if it == 6:
                    nc.scalar.activation(tmpq[:, 0, :], ssum[:, 0, :],
                                         AF.Sqrt, bias=eps64[0:2, :],
                                         scale=1.0)
                if it == 7:
                    with nc.allow_low_precision(reason="rq in bf16 for mm"):
                        nc.vector.reciprocal(rqb[:, 0, :], tmpq[:, 0, :])
                emit_proj(mt, ch, act_ok=True)
            while pend:
                emit_sums(*pend.pop(0))
            emit_khalf(0)
            emit_qscale(0)

            # ---- phase E: v + gate projection (fp8 DoubleRow)
            for t in range(ST):
                pvf = proj_psum()
                for ktp in range(KTP):
                    st = (ktp == 0)
                    sp = (ktp == KTP - 1)
                    nc.tensor.matmul(pvf(260),
                                     xts[:, ktp, :, 128 * t:128 * t + 128],
                                     wvgs[:, ktp, :, 0:260],
                                     start=st, stop=sp, perf_mode=DR)
                    nc.tensor.matmul(pot[:, 0, 4 * t:4 * t + 4],
                                     xts[:, ktp, :, 128 * t:128 * t + 128],
                                     wvgs[:, ktp, :, 256:260],
                                     start=st, stop=sp, perf_mode=DR)
                vdst = vbuf[:, t // 2, t % 2, :, 0:64]
                nc.vector.tensor_scalar(vdst, pvf(256),
                                        rvs[:, t:t + 1], None, op0=ALU.mult)
            nc.vector.tensor_tensor(graw[:], pot[:, 0, 0:64], rv4s[:],
                                    ALU.mult)
            nc.scalar.activation(gnat[:], graw[:], AF.Sigmoid)

            # ---- phase C part 2: mt3 (k heads 2/3) + mt1 (q heads 2/3).
            # All elementwise goes to the DVE: it drains during head-0/1
            # attention while the ACT is saturated with exp. The norm chains
            # for these heads are emitted inside the attention pipeline.
            for mt, ch in [(m, c) for m in (3, 1) for c in range(4)]:
                emit_proj(mt, ch, act_ok=False)
            while pend:
                emit_sums(*pend.pop(0))
            emit_khalf(1)

            # ---- attention, software-pipelined over heads:
            # cycle jc: scores+exp for head jc interleaved with the finalize
            # of head jc-1 (gated copy, transpose into drained po slot,
            # 4-batched copyback to obuf); attnV for head jc runs as one
            # block at the end (ex tiles buffered in a deep ring), after all
            # of head jc-1's transposes, so the po banks swap owners cleanly.
            # DVE exp ladder: exp(t) ~= (1+t/64)^64 via 6 squarings (f32 for
            # the first three, bf16 after; the DVE runs 2-byte sbuf ops at
            # 2-4x). ~6x the ACT cost per tile, but it spends otherwise-idle
            # DVE cycles to shave the ACT-bound attention phase.
            def emit_exp_dve(ps, dst, c):
                e0 = work.tile([P, 1024], F32, tag="e0")
                nc.vector.tensor_scalar(e0[:], ps[:], rkb64[:, c:c + 1],
                                        C1EXP, op0=ALU.mult, op1=ALU.add)
                nc.vector.tensor_tensor(e0[:], e0[:], e0[:], ALU.mult)
                nc.vector.tensor_tensor(e0[:], e0[:], e0[:], ALU.mult)
                e1 = work.tile([P, 1024], BF16, tag="e1")
                with nc.allow_low_precision(reason="softmax wts are fp8"):
                    nc.vector.tensor_tensor(e1[:], e0[:], e0[:], ALU.mult)
                    nc.vector.tensor_tensor(e1[:], e1[:], e1[:], ALU.mult)
                    nc.vector.tensor_tensor(e1[:], e1[:], e1[:], ALU.mult)
                    nc.vector.tensor_tensor(dst, e1[:], e1[:], ALU.mult)

            exts = {}
            # attnV emission schedule: (sktp, q) lands at the first loop
            # index where its ex pair is computed AND po slot q's batch has
            # been copied back to obuf for the previous head (tp of slot q
            # precedes it in the PE FIFO, so no cross-engine deadlock).
            avsched = {}
            for sktp_ in range(8):
                for q_ in range(16):
                    avsched.setdefault(
                        max(2 * sktp_ + 1, 4 * (q_ // 4) + 3), []).append(
                            (sktp_, q_))
            for v_ in avsched.values():
                v_.sort()
            for jc in range(5):
                j = jc if jc < 4 else None
                jf = jc - 1 if jc > 0 else None
                if jc == 1:
                    # mt1 q-norm sqrt lands after head-0's exps on the ACT
                    nc.scalar.activation(tmpq[:, 1, :], ssum[:, 1, :],
                                         AF.Sqrt, bias=eps64[0:2, :],
                                         scale=1.0)
                if jc == 2:
                    with nc.allow_low_precision(reason="rq in bf16 for mm"):
                        nc.vector.reciprocal(rqb[:, 1, :], tmpq[:, 1, :])
                    emit_qscale(1, late=True)
                if jf is not None:
                    # rs = 1/sums; crb = 32*gate*rs  (32 = fp8 range scale)
                    nc.vector.reciprocal(rs[:], pot[:, :, 64:512:128])
                    nc.vector.tensor_tensor(crb[:], rs[:], gnat[:, :, jf],
                                            ALU.mult)
                    nc.vector.tensor_scalar(crb[:], crb[:], 32.0, None,
                                            op0=ALU.mult)
                for skt in range(ST):
                    if j is not None:
                        mtq, mtk = j // 2, 2 + j // 2
                        a = 64 * (j % 2)
                        if skt % 2 == 0:
                            exts[(j, skt // 2)] = wex.tile(
                                [P, 2, S], F8, tag="ex", name="ext")
                        ext = exts[(j, skt // 2)]
                        for h in range(2):
                            ps = psc.tile([P, 1024], F32, tag="sc")
                            for c2 in range(2):
                                q0 = 1024 * h + 512 * c2
                                nc.tensor.matmul(
                                    ps[:, 512 * c2:512 * c2 + 512],
                                    qkT[a:a + 64, mtk,
                                        128 * skt:128 * skt + 128],
                                    qkT[a:a + 64, mtq, q0:q0 + 512],
                                    start=True, stop=True)
                            col = 32 * (j // 2) + 2 * skt + (j % 2)
                            dst = ext[:, skt % 2, 1024 * h:1024 * h + 1024]
                            if EXP_ACT_OF_32 < 32 and (2 * skt + h) % 8 == 7:
                                emit_exp_dve(ps, dst, col)
                            else:
                                nc.scalar.activation(
                                    dst, ps[:], AF.Exp, bias=expbb[:],
                                    scale=rkb[:, col:col + 1])
                    if jf is not None:
                        q = skt
                        posb = wsb.tile([P, 64], F32, tag="posb")
                        qo = 128 * (q % 4)
                        src = pot[:, q // 4, qo:qo + 64]
                        if j is None and q % 2 == 0:
                            nc.scalar.activation(posb[:], src, AF.Copy,
                                                 scale=crb[:, q:q + 1])
                        else:
                            nc.vector.tensor_scalar(posb[:], src,
                                                    crb[:, q:q + 1], None,
                                                    op0=ALU.mult)
                        # transpose into the just-drained po slot
                        nc.tensor.transpose(
                            pot[0:64, q // 4, qo:qo + 128],
                            posb[:], ident[:])
                        if q % 4 == 3:
                            b = q // 4
                            dst = obuf[:, jf, 512 * b:512 * b + 512]
                            srcq = pot[0:64, b, 0:512]
                            if j is None and b % 2 == 1:
                                nc.scalar.activation(dst, srcq, AF.Copy)
                            else:
                                nc.vector.tensor_copy(out=dst, in_=srcq)
                    if j is not None:
                        for sktp, q in avsched.get(skt, []):
                            nc.tensor.matmul(
                                pot[:, q // 4,
                                    128 * (q % 4):128 * (q % 4) + 65],
                                exts[(j, sktp)][:, :, 128 * q:128 * q + 128],
                                vbuf[:, sktp, :, j, 0:65],
                                start=(sktp == 0), stop=(sktp == 7),
                                perf_mode=DR)
                    if jf == 3 and skt % 4 == 3:
                        # o_proj for the s-range whose obuf batch just landed
                        for t in range(skt - 3, skt + 1):
                            ot = wot.tile([P, 1024], BF16, tag="ot")
                            for nh in range(2):
                                pp = psc.tile([P, 1024], F32, tag="sc")
                                for jp in range(2):
                                    nc.tensor.matmul(
                                        pp[:, 0:512],
                                        obuf[:, 2 * jp:2 * jp + 2,
                                             128 * t:128 * t + 128],
                                        wos[:, 2 * jp:2 * jp + 2,
                                            512 * nh:512 * nh + 512],
                                        start=(jp == 0), stop=(jp == 1),
                                        perf_mode=DR)
                                osl = ot[:, 512 * nh:512 * nh + 512]
                                if nh == 0:
                                    nc.scalar.activation(
                                        osl, pp[:, 0:512], AF.Copy,
                                        scale=2.0 ** -10)
                                else:
                                    nc.vector.tensor_scalar(
                                        osl, pp[:, 0:512], 2.0 ** -10,
                                        None, op0=ALU.mult)
                            if t % 2 == 0:
                                nc.sync.dma_start(
                                    out=outp[128 * t:128 * t + 128, :],
                                    in_=ot[:])
                            else:
                                nc.gpsimd.dma_start(
                                    out=outp[128 * t:128 * t + 128, :],
                                    in_=ot[:])
                if j is not None:
                    for sktp in range(8):
                        exts.pop((j, sktp))
    return nc


def _get_program():
    global _NC_CACHE
    if _NC_CACHE is None:
        _NC_CACHE = _build_program()
    return _NC_CACHE


# ----------------------------------------------------------------------------
# host wrapper
# ----------------------------------------------------------------------------

def _prep_inputs(x, prenorm_w, qkv_w, gate_w, o_w, q_norm_w, k_norm_w):
    x = np.asarray(x, np.float32)
    pw = np.asarray(prenorm_w, np.float32)
    qkv_w = np.asarray(qkv_w, np.float32)
    gate_w = np.asarray(gate_w, np.float32)
    o_w = np.asarray(o_w, np.float32)
    qw = qkv_w[0:D] * pw[None, :]
    kw = qkv_w[D:2 * D] * pw[None, :]
    vw = qkv_w[2 * D:3 * D] * pw[None, :]
    gw = gate_w * pw[None, :]

    r = 1.0 / np.sqrt(np.mean(x * x, axis=-1) + EPS)      # [B, S]
    ind = np.zeros((P, 2), BF)
    ind[0:64, 0] = 1
    ind[64:128, 1] = 1
    ind2 = np.zeros((2, P), BF)
    ind2[0, 0:64] = 1
    ind2[1, 64:128] = 1
    qn = (np.tile(np.asarray(q_norm_w, np.float32), 2) / 32.0)[:, None]
    kn = (np.tile(np.asarray(k_norm_w, np.float32), 2) / 32.0)[:, None]

    in_maps = []
    for c in range(8):
        b, hg = c // 4, c % 4
        hsl = slice(256 * hg, 256 * hg + 256)
        xtc = np.ascontiguousarray(
            x[b].T.reshape(KTP, 2, P, S).transpose(2, 0, 1, 3)).astype(F8NP)
        wqk = np.concatenate([qw[hsl], kw[hsl]], 0).T * 32.0  # [1024, 512]
        wqkc = np.ascontiguousarray(
            wqk.reshape(KTP, 2, P, 512).transpose(2, 0, 1, 3)).astype(F8NP)
        wvg = np.concatenate([vw[hsl], gw[4 * hg:4 * hg + 4]], 0).T * 32.0
        wvgp = np.zeros((D, 272), np.float32)
        wvgp[:, 0:260] = wvg
        wvgc = np.ascontiguousarray(
            wvgp.reshape(KTP, 2, P, 272).transpose(2, 0, 1, 3)).astype(F8NP)
        wo = o_w[:, hsl].T.reshape(NH, 64, D).transpose(1, 0, 2) * 32.0
        woc = np.ascontiguousarray(wo).astype(F8NP)
        rvc = np.ascontiguousarray(
            r[b].reshape(ST, P).T / 32.0).astype(np.float32)
        rv4c = np.ascontiguousarray(np.repeat(rvc, 4, axis=1))
        in_maps.append({
            "xt8": xtc, "wqk8": wqkc, "wvg8": wvgc, "wo8": woc,
            "rv": rvc, "rv4": rv4c, "qn": qn.astype(np.float32),
            "kn": kn.astype(np.float32), "ind": ind, "ind2": ind2,
        })
    return in_maps


_RUNNER = None


def _get_runner():
    """Build the sharded PJRT executable ONCE and reuse it across calls
    (run_bass_kernel_spmd re-traces/re-compiles on every invocation)."""
    global _RUNNER
    if _RUNNER is not None:
        return _RUNNER
    import jax
    import concourse.mybir as _mybir
    from concourse.bass2jax import (_bass_exec_p, partition_id_tensor,
                                    install_neuronx_cc_hook, Mesh,
                                    PartitionSpec, shard_map)
    install_neuronx_cc_hook()
    nc = _get_program()
    n_cores = 8
    partition_name = (nc.partition_id_tensor.name
                      if nc.partition_id_tensor else None)
    in_names, out_names, out_avals, zero_outs = [], [], [], []
    for alloc in nc.m.functions[0].allocations:
        if not isinstance(alloc, _mybir.MemoryLocationSet):
            continue
        name = alloc.memorylocations[0].name
        if alloc.kind == "ExternalInput":
            if name != partition_name:
                in_names.append(name)
        elif alloc.kind == "ExternalOutput":
            shape = tuple(alloc.tensor_shape)
            dtype = _mybir.dt.np(alloc.dtype)
            out_names.append(name)
            out_avals.append(jax.core.ShapedArray(shape, dtype))
            zero_outs.append(np.zeros(shape, dtype))
    n_params = len(in_names)
    n_outs = len(out_avals)
    all_in = list(in_names) + list(out_names)
    if partition_name is not None:
        all_in.append(partition_name)
    donate = tuple(range(n_params, n_params + n_outs))

    def _body(*args):
        operands = list(args)
        if partition_name is not None:
            operands.append(partition_id_tensor())
        return tuple(_bass_exec_p.bind(
            *operands, out_avals=tuple(out_avals), in_names=tuple(all_in),
            out_names=tuple(out_names), lowering_input_output_aliases=(),
            sim_require_finite=True, sim_require_nnan=True, nc=nc))

    devices = jax.devices()[:n_cores]
    mesh = Mesh(np.asarray(devices), ("core",))
    sharded = jax.jit(
        shard_map(_body, mesh=mesh,
                  in_specs=(PartitionSpec("core"),) * (n_params + n_outs),
                  out_specs=(PartitionSpec("core"),) * n_outs,
                  check_rep=False),
        donate_argnums=donate, keep_unused=True)
    _RUNNER = (sharded, in_names, out_names, out_avals, zero_outs, n_cores)
    return _RUNNER


def kernel(x, prenorm_w, qkv_w, gate_w, o_w, q_norm_w, k_norm_w):
    sharded, in_names, out_names, out_avals, zero_outs, n_cores = _get_runner()
    in_maps = _prep_inputs(x, prenorm_w, qkv_w, gate_w, o_w,
                           q_norm_w, k_norm_w)
    concat_in = [np.concatenate([in_maps[c][nm] for c in range(n_cores)], 0)
                 for nm in in_names]
    concat_zeros = [np.zeros((n_cores * z.shape[0], *z.shape[1:]), z.dtype)
                    for z in zero_outs]
    out_arrs = sharded(*concat_in, *concat_zeros)
    oi = out_names.index("out_p")
    op = np.asarray(out_arrs[oi]).astype(np.float32).reshape(
        n_cores, *out_avals[oi].shape)
    outs = [op[c] for c in range(n_cores)]
    x = np.asarray(x, np.float32)
    y0 = x[0] + outs[0] + outs[1] + outs[2] + outs[3]
    y1 = x[1] + outs[4] + outs[5] + outs[6] + outs[7]
    return np.stack([y0, y1]).astype(np.float32)


# revision 49
# speedup vs baseline: 1.5834x; 1.0794x over previous
"""GatedAttention Trainium2 kernel (8 NeuronCores, tensor-parallel over (batch, head-group)).

Sharding: core c handles batch b=c//4 and heads 4*(c%4)..4*(c%4)+3.
Each core computes qkv/gate projections for its heads from x[b], per-head
QK-RMS-norm + softmax attention + sigmoid gating, and a row-split o_proj
partial [S, D] (bf16). Host sums the 4 partials per batch + residual.

Key structure:
- All big matmuls run in fp8e4m3 with DoubleRow (contraction pairs packed
  into the free dim of both operands: [Ki, 2, M] x [Ki, 2, N]). Weights are
  pre-scaled x32 on host so fp8 sees ~unit-variance values; descales fold
  into copyback scales.
- attnV emits [sq, hd] tiles (out free = 65) instead of [hd, sq]: softmax
  sums + sigmoid gate become per-partition scalars. A per-sq-tile PE
  transpose rebuilds the [hd, sq] layout o_proj needs, writing into the
  just-drained po PSUM slot.
- exp is the elementwise wall (16.8M elems/core): batched to N=1024 per
  instruction and split between ACT (native Exp) and a custom single-pass
  8-stage DVE op computing exp(t) ~= (1 + t/64)^64.
- softmax runs without max-subtraction; a uniform -3*ln2 bias keeps exp
  outputs under the fp8e4m3 max (e^(8-2.079)=372<448; |s|<=8 by
  Cauchy-Schwarz after QK RMS norm). The bias cancels in normalization.
- v carries an all-ones column so attnV also yields the softmax sums.
"""

import json
import math

import numpy as np
import ml_dtypes

import concourse.bass as bass
import concourse.bass_utils as bass_utils
import concourse.bass2jax as bass2jax
import concourse.mybir as mybir
import concourse.tile as tile
import concourse.dve_ops as dve_ops
from concourse.dve_ops import DveOp
from concourse.dve_spec import Spec, Src0, C0, C1, sq as dve_sq
from concourse.tile import TileContext
from concourse.masks import make_identity
from concourse.vector_clock import ScopedClock, VectorClock

F32 = mybir.dt.float32
BF16 = mybir.dt.bfloat16
F8 = mybir.dt.float8e4
AF = mybir.ActivationFunctionType
ALU = mybir.AluOpType
DR = mybir.MatmulPerfMode.DoubleRow
BF = ml_dtypes.bfloat16
F8NP = mybir.dt.np(mybir.dt.float8e4)

B, S, D = 2, 2048, 1024
NH_TOT, HD = 16, 64
NH = 4            # heads per core
EPS = 1e-5
P = 128
ST = S // P       # 16 s-tiles
KTP = 4           # d-dim pair-tiles (4 x (2x128))
EXPB = 3 * math.log(2.0)          # uniform score bias (cancels in softmax)
C1EXP = 1.0 - EXPB / 64.0         # dve exp: a = s*rk/64 + C1EXP

# engine split knobs (tuned against sim engine-busy readout)
# NOTE: this container's walrus cannot codegen custom-DVE ops ("ISA wrong
# length" even for the production RECIPROCAL_APPROX_FAST), so exp runs
# entirely on ACT and everything else moves to DVE.
EXP_ACT_OF_32 = 32   # of the 32 exp half-tiles per head, this many on ACT

# ----------------------------------------------------------------------------
# custom DVE op: exp(t) ~= (1 + t/64)^64, one pass, 8 uop stages
# ----------------------------------------------------------------------------


def _ref_exp64(in0, in1, s0, s1, imm2):
    a = in0.astype(np.float32) * np.asarray(s0, np.float32) + np.float32(s1)
    for _ in range(6):
        a = a * a
    return a


EXP64_ANT = DveOp(
    "EXP64_ANT",
    Spec(
        body=dve_sq(dve_sq(dve_sq(dve_sq(dve_sq(dve_sq(Src0 * C0 + C1)))))),
        reference=_ref_exp64,
    ),
    subdim=False,
    uops_sha={"v3": "8299cc4e9a89acf1", "v4": "df7b3d1456faeb1a"},
)


def _register_exp_op():
    if EXP64_ANT.name in dve_ops.CUSTOM_DVE_SPECS:
        return
    row = max(dve_ops._SUB_OPCODE_FOR_NAME.values()) + 1
    assert row < 0x20
    dve_ops.OPS.append(EXP64_ANT)
    dve_ops.CUSTOM_DVE_SPECS[EXP64_ANT.name] = EXP64_ANT.spec
    dve_ops._SUB_OPCODE_FOR_NAME[EXP64_ANT.name] = row


_register_exp_op()

# ----------------------------------------------------------------------------
# compat patches: this walrus build accepts only ONE sync-wait per instruction
# ----------------------------------------------------------------------------

def _patched_drain_and_barrier(self, tick_clock, wait_clock):
    nc = self.nc
    gc = tick_clock.global_clock
    n = len(gc)
    for p in range(n):
        t = gc[p]
        if t <= 0:
            continue
        vec = VectorClock([0] * n)
        vec.require_at_least(p, t)
        nop = nc.sync.nop(nofuse=True, hint=f"drain_wait_p{p}")
        wait_clock.add_sem_waits(nop.ins, ScopedClock({None: vec}))
    nc.sync.drain(fusable=False)
    nc.all_engine_barrier()
    assert self.sems is not None
    popped = nc._tile_sem_poison_stack.pop()
    assert popped is self._sem_poison
    nc.clear_and_free_semaphores(list(self.sems.allocated().values()))
    nc.all_engine_barrier()


def _split_multi_waits(bir_json: bytes) -> bytes:
    bj = json.loads(bir_json)
    n_split = 0
    for fn in bj.get("functions", []):
        for blk in fn.get("blocks", []):
            out = []
            for inst in blk.get("instructions", []):
                si = inst.get("sync_info")
                waits = si.get("on_wait", []) if si else []
                if len(waits) > 1:
                    for i, w in enumerate(waits[:-1]):
                        out.append({
                            "debug": inst.get("debug"),
                            "engine": inst["engine"],
                            "ins": [], "outs": [],
                            "name": f"{inst['name']}_sw{i}",
                            "opcode": "NoOp",
                            "sync_info": {"on_update": [], "on_wait": [w]},
                            "text_hint": "split_wait",
                        })
                        n_split += 1
                    si["on_wait"] = [waits[-1]]
                out.append(inst)
            blk["instructions"] = out
    if n_split:
        return json.dumps(bj).encode()
    return bir_json


_ORIG_COMPILE = bass_utils.compile_bir_kernel


def _patched_compile_bir_kernel(bir_json, tmpdir, neff_name="file.neff"):
    return _ORIG_COMPILE(_split_multi_waits(bir_json), tmpdir, neff_name)


def _apply_compat():
    tile.TileContext._drain_and_barrier = _patched_drain_and_barrier
    bass_utils.compile_bir_kernel = _patched_compile_bir_kernel
    bass2jax.compile_bir_kernel = _patched_compile_bir_kernel


_apply_compat()

# ----------------------------------------------------------------------------
# device program (SPMD: identical program, per-core data)
# ----------------------------------------------------------------------------

_NC_CACHE = None


def _build_program():
    nc = bass.Bass()
    xt8 = nc.declare_dram_parameter("xt8", [P, KTP, 2, S], F8, isOutput=False)
    wqk8 = nc.declare_dram_parameter("wqk8", [P, KTP, 2, 512], F8, isOutput=False)
    wvg8 = nc.declare_dram_parameter("wvg8", [P, KTP, 2, 272], F8, isOutput=False)
    wo8 = nc.declare_dram_parameter("wo8", [64, NH, D], F8, isOutput=False)
    rv = nc.declare_dram_parameter("rv", [P, ST], F32, isOutput=False)
    rv4 = nc.declare_dram_parameter("rv4", [P, 64], F32, isOutput=False)
    qn = nc.declare_dram_parameter("qn", [P, 1], F32, isOutput=False)
    kn = nc.declare_dram_parameter("kn", [P, 1], F32, isOutput=False)
    ind = nc.declare_dram_parameter("ind", [P, 2], BF16, isOutput=False)
    ind2 = nc.declare_dram_parameter("ind2", [2, P], BF16, isOutput=False)
    outp = nc.declare_dram_parameter("out_p", [S, D], BF16, isOutput=True)

    with TileContext(nc) as tc:
        with tc.tile_pool(name="big", bufs=1) as big, \
             tc.tile_pool(name="work", bufs=4) as work, \
             tc.tile_pool(name="wex", bufs=9) as wex, \
             tc.tile_pool(name="wsb", bufs=4) as wsb, \
             tc.tile_pool(name="wot", bufs=4) as wot, \
             tc.tile_pool(name="psc", bufs=2, space="PSUM") as psc, \
             tc.tile_pool(name="ppo", bufs=1, space="PSUM") as ppo:

            # ---- resident inputs (weights first; x chunked by s-range so
            # the first projection iterations start early; wo8 needed last)
            wqks = big.tile([P, KTP, 2, 512], F8)
            nc.sync.dma_start(out=wqks[:], in_=wqk8[:, :, :, :])
            xts = big.tile([P, KTP, 2, S], F8)
            for xc in range(4):
                nc.sync.dma_start(
                    out=xts[:, :, :, 512 * xc:512 * xc + 512],
                    in_=xt8[:, :, :, 512 * xc:512 * xc + 512])
            wvgs = big.tile([P, KTP, 2, 272], F8)
            nc.sync.dma_start(out=wvgs[:], in_=wvg8[:, :, :, :])
            wos = big.tile([64, NH, D], F8)
            nc.gpsimd.dma_start(out=wos[:], in_=wo8[:, :, :])
            rvs = big.tile([P, ST], F32)
            nc.sync.dma_start(out=rvs[:], in_=rv[:, :])
            rv4s = big.tile([P, 64], F32)
            nc.sync.dma_start(out=rv4s[:], in_=rv4[:, :])
            qns = big.tile([P, 1], F32)
            nc.sync.dma_start(out=qns[:], in_=qn[:, :])
            kns = big.tile([P, 1], F32)
            nc.sync.dma_start(out=kns[:], in_=kn[:, :])
            inds = big.tile([P, 2], BF16)
            nc.sync.dma_start(out=inds[:], in_=ind[:, :])
            ind2s = big.tile([2, P], BF16)
            nc.sync.dma_start(out=ind2s[:], in_=ind2[:, :])

            ident = big.tile([P, P], F32)
            make_identity(nc, ident[:])
            epsb = big.tile([P, 1], F32)
            nc.vector.memset(epsb[:], EPS)
            eps64 = big.tile([P, 1], F32)
            nc.vector.memset(eps64[:], HD * EPS)
            expbb = big.tile([P, 1], F32)
            nc.vector.memset(expbb[:], -EXPB)

            # ---- resident intermediates
            qkT = big.tile([P, 4, S], BF16)          # mt0,1=q(pair0,1) mt2,3=k
            vbuf = big.tile([P, ST // 2, 2, NH, 68], F8)  # [sk%128,sktp,e,j,hd+1]
            nc.vector.memset(vbuf[:, :, :, :, 64:65], 1.0)
            obuf = big.tile([64, NH, S], F8)         # gated attn outT per head
            gnat = big.tile([P, 16, 4], F32)         # tanh(r*gate/2) [t, j]
            gnat3 = big.tile([P, 16, 4], F32)        # 16*(1+tanh) = 32*sigmoid
            gstage = big.tile([P, 16, 4], F32)       # raw gate rows
            rkb = big.tile([P, 64], F32)             # 1/rms(k), col=4skt+j
            rkb64 = big.tile([P, 64], F32)           # rkb/64 (dve exp scale)
            ssum = big.tile([2, 2, S], F32)          # q sumsq rows per mt
            rqb = big.tile([2, 2, S], BF16)          # q scale rows per mt
            tmpq = big.tile([2, 2, S], F32)
            tmpk = big.tile([P, 64], F32)
            graw = big.tile([P, 64], F32)
            rs = big.tile([P, 16], F32)              # recip softmax sums
            crb = big.tile([P, 16], F32)             # 32*gate*rs per sq-tile

            # po layout: [bank(4), slot(4), col(128)]; slice q -> [q//4, q%4]
            # pre-attention scratch carved from the same banks:
            #   gate psum = pot[:, 0, 0, 0:64]; k-sumsq psum = pot[:, 0, 1, 0:64]
            pot = ppo.tile([P, 4, 512], F32, tag="po")

            # k-norm column layout: kcol(skt, j) = 32*(j//2) + 2*skt + (j%2)
            # so each k head-pair's stats occupy one contiguous 32-col half.
            def emit_sums(mt, ch, sqt):
                if mt < 2:  # q: row-layout sums [2, 512]
                    prf = proj_psum()
                    nc.tensor.matmul(prf(512)[0:2], inds[:], sqt[:],
                                     start=True, stop=True)
                    nc.vector.tensor_copy(
                        out=ssum[:, mt, 512 * ch:512 * ch + 512],
                        in_=prf(512)[0:2])
                else:  # k: column sums
                    for sl in range(4):
                        skt = ch * 4 + sl
                        c0 = 128 + 32 * (mt - 2) + 2 * skt
                        nc.tensor.matmul(
                            pot[:, 0, c0:c0 + 2],
                            sqt[:, 128 * sl:128 * sl + 128], inds[:],
                            start=True, stop=True)

            # projection iteration; elementwise on ACT only where allowed
            # (anything on the ACT FIFO ahead of the exps delays attention).
            # Projection psum rotates over 4 slots: the 2-deep "sc" ring plus
            # pot banks 2/3, which attention doesn't touch until its q>=8
            # attnV writes (far later, ordered by the tile framework).
            pend = []
            pslot = [0]

            def proj_psum():
                s = pslot[0] = 2 + (pslot[0] + 1) % 2
                return lambda n, s=s: pot[:, s, 0:n]

            def emit_proj(mt, ch, act_ok):
                if act_ok:
                    pqt = psc.tile([P, 1024], F32, tag="sc", name="pqt")
                    pqf = lambda n: pqt[:, 0:n]
                else:
                    pqf = proj_psum()
                for ktp in range(KTP):
                    nc.tensor.matmul(
                        pqf(512),
                        wqks[:, ktp, :, 128 * mt:128 * mt + 128],
                        xts[:, ktp, :, 512 * ch:512 * ch + 512],
                        start=(ktp == 0), stop=(ktp == KTP - 1),
                        perf_mode=DR)
                sc = qns if mt < 2 else kns
                qsl = qkT[:, mt, 512 * ch:512 * ch + 512]
                if act_ok:
                    nc.scalar.activation(qsl, pqf(512), AF.Copy,
                                         scale=sc[:])
                else:
                    nc.vector.tensor_scalar(qsl, pqf(512), sc[:],
                                            None, op0=ALU.mult)
                # squares from the bf16 copy (sbuf 2-byte: fast DVE path)
                sqt = work.tile([P, 512], BF16, tag="sq")
                nc.vector.tensor_tensor(sqt[:], qsl, qsl, ALU.mult)
                pend.append((mt, ch, sqt))
                if len(pend) > 2:
                    emit_sums(*pend.pop(0))

            def emit_khalf(half):
                sl = slice(32 * half, 32 * half + 32)
                psl = slice(128 + 32 * half, 160 + 32 * half)
                nc.scalar.activation(tmpk[:, sl], pot[:, 0, psl],
                                     AF.Sqrt, bias=epsb[:], scale=1.0 / HD)
                nc.vector.reciprocal(rkb[:, sl], tmpk[:, sl])
                nc.vector.tensor_scalar(rkb64[:, sl], rkb[:, sl],
                                        1.0 / 64.0, None, op0=ALU.mult)

            def emit_qscale(mt, late=False):
                for chq in range(4):
                    if late:
                        pbct = psc.tile([P, 1024], F32, tag="sc", name="pbct")
                        pbc = lambda n: pbct[:, 0:n]
                    else:
                        pbc = proj_psum()
                    nc.tensor.matmul(
                        pbc(512), ind2s[:],
                        rqb[:, mt, 512 * chq:512 * chq + 512],
                        start=True, stop=True)
                    nc.vector.tensor_tensor(
                        qkT[:, mt, 512 * chq:512 * chq + 512],
                        qkT[:, mt, 512 * chq:512 * chq + 512],
                        pbc(512), ALU.mult)

            # ---- phase C part 1: mt0 (q heads 0/1) + mt2 (k heads 0/1),
            # interleaved by s-chunk to match the x DMA chunk arrival
            for mt, ch in [(m, c) for c in range(4) for m in (0, 2)]:
                emit_proj(mt, ch, act_ok=True)
            while pend:
                emit_sums(*pend.pop(0))
            nc.scalar.activation(tmpq[:, 0, :], ssum[:, 0, :], AF.Sqrt,
                                 bias=eps64[0:2, :], scale=1.0)
            with nc.allow_low_precision(reason="rq in bf16 for mm"):
                nc.vector.reciprocal(rqb[:, 0, :], tmpq[:, 0, :])
            emit_khalf(0)
            emit_qscale(0)

            # ---- phase E helper: v + gate projection (fp8 DoubleRow).
            # Emitted inside attention cycle 0, hidden under head-0's exps.
            def emit_E(t):
                pvf = proj_psum()
                for ktp in range(KTP):
                    nc.tensor.matmul(pvf(260),
                                     xts[:, ktp, :, 128 * t:128 * t + 128],
                                     wvgs[:, ktp, :, 0:260],
                                     start=(ktp == 0), stop=(ktp == KTP - 1),
                                     perf_mode=DR)
                vdst = vbuf[:, t // 2, t % 2, :, 0:64]
                nc.vector.tensor_scalar(vdst, pvf(260)[0:P, 0:256],
                                        rvs[:, t:t + 1], None, op0=ALU.mult)
                nc.vector.tensor_copy(out=gstage[:, t, :],
                                      in_=pvf(260)[0:P, 256:260])

            # ---- phase C part 2: mt3 (k heads 2/3) + mt1 (q heads 2/3).
            # All elementwise goes to the DVE: it drains during head-0/1
            # attention while the ACT is saturated with exp. The norm chains
            # for these heads are emitted inside the attention pipeline.
            for mt, ch in [(m, c) for m in (3, 1) for c in range(4)]:
                emit_proj(mt, ch, act_ok=False)
            while pend:
                emit_sums(*pend.pop(0))
            emit_khalf(1)
            nc.scalar.activation(tmpq[:, 1, :], ssum[:, 1, :], AF.Sqrt,
                                 bias=eps64[0:2, :], scale=1.0)
            with nc.allow_low_precision(reason="rq in bf16 for mm"):
                nc.vector.reciprocal(rqb[:, 1, :], tmpq[:, 1, :])
            emit_qscale(1)

            # ---- attention, software-pipelined over heads:
            # cycle jc: scores+exp for head jc interleaved with the finalize
            # of head jc-1 (gated copy, transpose into drained po slot,
            # 4-batched copyback to obuf); attnV for head jc runs as one
            # block at the end (ex tiles buffered in a deep ring), after all
            # of head jc-1's transposes, so the po banks swap owners cleanly.
            # DVE exp ladder: exp(t) ~= (1+t/64)^64 via 6 squarings (f32 for
            # the first three, bf16 after; the DVE runs 2-byte sbuf ops at
            # 2-4x). ~6x the ACT cost per tile, but it spends otherwise-idle
            # DVE cycles to shave the ACT-bound attention phase.
            def emit_exp_dve(ps, dst, c):
                e0 = work.tile([P, 1024], F32, tag="e0")
                nc.vector.tensor_scalar(e0[:], ps[:], rkb64[:, c:c + 1],
                                        C1EXP, op0=ALU.mult, op1=ALU.add)
                nc.vector.tensor_tensor(e0[:], e0[:], e0[:], ALU.mult)
                nc.vector.tensor_tensor(e0[:], e0[:], e0[:], ALU.mult)
                e1 = work.tile([P, 1024], BF16, tag="e1")
                with nc.allow_low_precision(reason="softmax wts are fp8"):
                    nc.vector.tensor_tensor(e1[:], e0[:], e0[:], ALU.mult)
                    nc.vector.tensor_tensor(e1[:], e1[:], e1[:], ALU.mult)
                    nc.vector.tensor_tensor(e1[:], e1[:], e1[:], ALU.mult)
                    nc.vector.tensor_tensor(dst, e1[:], e1[:], ALU.mult)

            exts = {}
            # attnV emission schedule: (sktp, q) lands at the first loop
            # index where its ex pair is computed AND po slot q's batch has
            # been copied back to obuf for the previous head (tp of slot q
            # precedes it in the PE FIFO, so no cross-engine deadlock).
            avsched = {}
            for sktp_ in range(8):
                for q_ in range(16):
                    avsched.setdefault(
                        max(2 * sktp_ + 1, 4 * (q_ // 4) + 3), []).append(
                            (sktp_, q_))
            for v_ in avsched.values():
                v_.sort()
            # cycle 0 variant: pot banks 2/3 double as phase-E psum slots, so
            # the q>=8 attnV (which overwrites them) waits for the last E tile
            avsched0 = {}
            for sktp_ in range(8):
                for q_ in range(16):
                    avsched0.setdefault(
                        15 if q_ >= 8 else max(2 * sktp_ + 1,
                                               4 * (q_ // 4) + 3), []).append(
                            (sktp_, q_))
            for v_ in avsched0.values():
                v_.sort()
            def emit_scores_exp(j, skt, h):
                mtq, mtk = j // 2, 2 + j // 2
                a = 64 * (j % 2)
                if (j, skt // 2) not in exts:
                    exts[(j, skt // 2)] = wex.tile(
                        [P, 2, S], F8, tag="ex", name="ext")
                ext = exts[(j, skt // 2)]
                ps = psc.tile([P, 1024], F32, tag="sc")
                for c2 in range(2):
                    q0 = 1024 * h + 512 * c2
                    nc.tensor.matmul(
                        ps[:, 512 * c2:512 * c2 + 512],
                        qkT[a:a + 64, mtk, 128 * skt:128 * skt + 128],
                        qkT[a:a + 64, mtq, q0:q0 + 512],
                        start=True, stop=True)
                col = 32 * (j // 2) + 2 * skt + (j % 2)
                nc.scalar.activation(
                    ext[:, skt % 2, 1024 * h:1024 * h + 1024], ps[:],
                    AF.Exp, bias=expbb[:], scale=rkb[:, col:col + 1])

            def emit_fin(jf, q, act_mix):
                posb = wsb.tile([P, 64], F32, tag="posb")
                qo = 128 * (q % 4)
                src = pot[:, q // 4, qo:qo + 64]
                if act_mix and q % 2 == 0:
                    nc.scalar.activation(posb[:], src, AF.Copy,
                                         scale=crb[:, q:q + 1])
                else:
                    nc.vector.tensor_scalar(posb[:], src, crb[:, q:q + 1],
                                            None, op0=ALU.mult)
                # transpose into the just-drained po slot
                nc.tensor.transpose(pot[0:64, q // 4, qo:qo + 128],
                                    posb[:], ident[:])
                if q % 4 == 3:
                    b = q // 4
                    dst = obuf[:, jf, 512 * b:512 * b + 512]
                    srcq = pot[0:64, b, 0:512]
                    if act_mix and b % 2 == 1:
                        nc.scalar.activation(dst, srcq, AF.Copy)
                    else:
                        nc.vector.tensor_copy(out=dst, in_=srcq)

            def emit_attnv(j, sktp, q):
                nc.tensor.matmul(
                    pot[:, q // 4, 128 * (q % 4):128 * (q % 4) + 65],
                    exts[(j, sktp)][:, :, 128 * q:128 * q + 128],
                    vbuf[:, sktp, :, j, 0:65],
                    start=(sktp == 0), stop=(sktp == 7), perf_mode=DR)

            def emit_oproj(t):
                ot = wot.tile([P, 1024], BF16, tag="ot")
                pp = psc.tile([P, 1024], F32, tag="sc")
                for nh in range(2):
                    for jp in range(2):
                        nc.tensor.matmul(
                            pp[:, 512 * nh:512 * nh + 512],
                            obuf[:, 2 * jp:2 * jp + 2,
                                 128 * t:128 * t + 128],
                            wos[:, 2 * jp:2 * jp + 2,
                                512 * nh:512 * nh + 512],
                            start=(jp == 0), stop=(jp == 1), perf_mode=DR)
                if t % 2 == 0:
                    nc.scalar.activation(ot[:], pp[:], AF.Copy,
                                         scale=2.0 ** -10)
                else:
                    nc.vector.tensor_scalar(ot[:], pp[:], 2.0 ** -10,
                                            None, op0=ALU.mult)
                if t % 2 == 0:
                    nc.sync.dma_start(out=outp[128 * t:128 * t + 128, :],
                                      in_=ot[:])
                else:
                    nc.gpsimd.dma_start(out=outp[128 * t:128 * t + 128, :],
                                        in_=ot[:])

            # heads 0-2: scores+exp of head j over the finalize of head j-1
            for jc in range(3):
                j = jc
                jf = jc - 1 if jc > 0 else None
                if jc == 1:
                    # gate: 32*sigmoid(rg) = 16*(1+tanh(rg/2)); Tanh shares
                    # the exp table set, so no ACT table reload here
                    nc.vector.tensor_tensor(graw[:], gstage[:], rv4s[:],
                                            ALU.mult)
                    nc.scalar.activation(gnat[:], graw[:], AF.Tanh,
                                         scale=0.5)
                    nc.vector.tensor_scalar(gnat3[:], gnat[:], 1.0, 16.0,
                                            op0=ALU.add, op1=ALU.mult)
                if jf is not None:
                    nc.vector.reciprocal(rs[:], pot[:, :, 64:512:128])
                    nc.vector.tensor_tensor(crb[:], rs[:], gnat3[:, :, jf],
                                            ALU.mult)
                for skt in range(ST):
                    emit_scores_exp(j, skt, 0)
                    emit_scores_exp(j, skt, 1)
                    if jf is not None:
                        emit_fin(jf, skt, False)
                    if jc == 0:
                        emit_E(skt)
                    sched = avsched0 if jc == 0 else avsched
                    for sktp, q in sched.get(skt, []):
                        emit_attnv(j, sktp, q)

            # head 3, pass A: h=0 exps cover attnV for sq-tiles 0-7;
            # head-2 finalize interleaves as usual
            nc.vector.reciprocal(rs[:], pot[:, :, 64:512:128])
            nc.vector.tensor_tensor(crb[:], rs[:], gnat3[:, :, 2],
                                    ALU.mult)
            for skt in range(ST):
                emit_scores_exp(3, skt, 0)
                emit_fin(2, skt, False)
                for sktp, q in avsched.get(skt, []):
                    if q < 8:
                        emit_attnv(3, sktp, q)
            # pass B: h=1 exps; head-3's low sq-tiles finalize and the first
            # half of o_proj runs underneath them
            nc.vector.reciprocal(rs[:, 0:8], pot[:, 0:2, 64:512:128])
            nc.vector.tensor_tensor(crb[:, 0:8], rs[:, 0:8],
                                    gnat3[:, 0:8, 3], ALU.mult)
            for idx in range(ST):
                emit_scores_exp(3, idx, 1)
                if idx % 2 == 0:
                    emit_fin(3, idx // 2, False)
                else:
                    for q in range(8, 16):
                        emit_attnv(3, (idx - 1) // 2, q)
                if idx >= 7 and idx % 2 == 1:
                    emit_oproj((idx - 7) // 2)
            # pass C: high sq-tiles of head 3 + the rest of o_proj
            nc.vector.reciprocal(rs[:, 8:16], pot[:, 2:4, 64:512:128])
            nc.vector.tensor_tensor(crb[:, 8:16], rs[:, 8:16],
                                    gnat3[:, 8:16, 3], ALU.mult)
            oq = [4], [5], [6], [7, 8], [9], [10], [11], [12, 13, 14, 15]
            for i, q in enumerate(range(8, 16)):
                emit_fin(3, q, True)
                for t in oq[i]:
                    emit_oproj(t)
    return nc


def _get_program():
    global _NC_CACHE
    if _NC_CACHE is None:
        _NC_CACHE = _build_program()
    return _NC_CACHE


# ----------------------------------------------------------------------------
# host wrapper
# ----------------------------------------------------------------------------

def _prep_inputs(x, prenorm_w, qkv_w, gate_w, o_w, q_norm_w, k_norm_w):
    x = np.asarray(x, np.float32)
    pw = np.asarray(prenorm_w, np.float32)
    qkv_w = np.asarray(qkv_w, np.float32)
    gate_w = np.asarray(gate_w, np.float32)
    o_w = np.asarray(o_w, np.float32)
    qw = qkv_w[0:D] * pw[None, :]
    kw = qkv_w[D:2 * D] * pw[None, :]
    vw = qkv_w[2 * D:3 * D] * pw[None, :]
    gw = gate_w * pw[None, :]

    r = 1.0 / np.sqrt(np.mean(x * x, axis=-1) + EPS)      # [B, S]
    ind = np.zeros((P, 2), BF)
    ind[0:64, 0] = 1
    ind[64:128, 1] = 1
    ind2 = np.zeros((2, P), BF)
    ind2[0, 0:64] = 1
    ind2[1, 64:128] = 1
    qn = (np.tile(np.asarray(q_norm_w, np.float32), 2) / 32.0)[:, None]
    kn = (np.tile(np.asarray(k_norm_w, np.float32), 2) / 32.0)[:, None]

    in_maps = []
    for c in range(8):
        b, hg = c // 4, c % 4
        hsl = slice(256 * hg, 256 * hg + 256)
        xtc = np.ascontiguousarray(
            x[b].T.reshape(KTP, 2, P, S).transpose(2, 0, 1, 3)).astype(F8NP)
        wqk = np.concatenate([qw[hsl], kw[hsl]], 0).T * 32.0  # [1024, 512]
        wqkc = np.ascontiguousarray(
            wqk.reshape(KTP, 2, P, 512).transpose(2, 0, 1, 3)).astype(F8NP)
        wvg = np.concatenate([vw[hsl], gw[4 * hg:4 * hg + 4]], 0).T * 32.0
        wvgp = np.zeros((D, 272), np.float32)
        wvgp[:, 0:260] = wvg
        wvgc = np.ascontiguousarray(
            wvgp.reshape(KTP, 2, P, 272).transpose(2, 0, 1, 3)).astype(F8NP)
        wo = o_w[:, hsl].T.reshape(NH, 64, D).transpose(1, 0, 2) * 32.0
        woc = np.ascontiguousarray(wo).astype(F8NP)
        rvc = np.ascontiguousarray(
            r[b].reshape(ST, P).T / 32.0).astype(np.float32)
        rv4c = np.ascontiguousarray(np.repeat(rvc, 4, axis=1))
        in_maps.append({
            "xt8": xtc, "wqk8": wqkc, "wvg8": wvgc, "wo8": woc,
            "rv": rvc, "rv4": rv4c, "qn": qn.astype(np.float32),
            "kn": kn.astype(np.float32), "ind": ind, "ind2": ind2,
        })
    return in_maps


_RUNNER = None


def _get_runner():
    """Build the sharded PJRT executable ONCE and reuse it across calls
    (run_bass_kernel_spmd re-traces/re-compiles on every invocation)."""
    global _RUNNER
    if _RUNNER is not None:
        return _RUNNER
    import jax
    import concourse.mybir as _mybir
    from concourse.bass2jax import (_bass_exec_p, partition_id_tensor,
                                    install_neuronx_cc_hook, Mesh,
                                    PartitionSpec, shard_map)
    install_neuronx_cc_hook()
    nc = _get_program()
    n_cores = 8
    partition_name = (nc.partition_id_tensor.name
                      if nc.partition_id_tensor else None)
    in_names, out_names, out_avals, zero_outs = [], [], [], []
    for alloc in nc.m.functions[0].allocations:
        if not isinstance(alloc, _mybir.MemoryLocationSet):
            continue
        name = alloc.memorylocations[0].name
        if alloc.kind == "ExternalInput":
            if name != partition_name:
                in_names.append(name)
        elif alloc.kind == "ExternalOutput":
            shape = tuple(alloc.tensor_shape)
            dtype = _mybir.dt.np(alloc.dtype)
            out_names.append(name)
            out_avals.append(jax.core.ShapedArray(shape, dtype))
            zero_outs.append(np.zeros(shape, dtype))
    n_params = len(in_names)
    n_outs = len(out_avals)
    all_in = list(in_names) + list(out_names)
    if partition_name is not None:
        all_in.append(partition_name)
    donate = tuple(range(n_params, n_params + n_outs))

    def _body(*args):
        operands = list(args)
        if partition_name is not None:
            operands.append(partition_id_tensor())
        return tuple(_bass_exec_p.bind(
            *operands, out_avals=tuple(out_avals), in_names=tuple(all_in),
            out_names=tuple(out_names), lowering_input_output_aliases=(),
            sim_require_finite=True, sim_require_nnan=True, nc=nc))

    devices = jax.devices()[:n_cores]
    mesh = Mesh(np.asarray(devices), ("core",))
    sharded = jax.jit(
        shard_map(_body, mesh=mesh,
                  in_specs=(PartitionSpec("core"),) * (n_params + n_outs),
                  out_specs=(PartitionSpec("core"),) * n_outs,
                  check_rep=False),
        donate_argnums=donate, keep_unused=True)
    _RUNNER = (sharded, in_names, out_names, out_avals, zero_outs, n_cores)
    return _RUNNER


def kernel(x, prenorm_w, qkv_w, gate_w, o_w, q_norm_w, k_norm_w):
    sharded, in_names, out_names, out_avals, zero_outs, n_cores = _get_runner()
    in_maps = _prep_inputs(x, prenorm_w, qkv_w, gate_w, o_w,
                           q_norm_w, k_norm_w)
    concat_in = [np.concatenate([in_maps[c][nm] for c in range(n_cores)], 0)
                 for nm in in_names]
    concat_zeros = [np.zeros((n_cores * z.shape[0], *z.shape[1:]), z.dtype)
                    for z in zero_outs]
    out_arrs = sharded(*concat_in, *concat_zeros)
    oi = out_names.index("out_p")
    op = np.asarray(out_arrs[oi]).astype(np.float32).reshape(
        n_cores, *out_avals[oi].shape)
    outs = [op[c] for c in range(n_cores)]
    x = np.asarray(x, np.float32)
    y0 = x[0] + outs[0] + outs[1] + outs[2] + outs[3]
    y1 = x[1] + outs[4] + outs[5] + outs[6] + outs[7]
    return np.stack([y0, y1]).astype(np.float32)


# revision 54
# speedup vs baseline: 1.6622x; 1.0498x over previous
"""GatedAttention Trainium2 kernel (8 NeuronCores, tensor-parallel over (batch, head-group)).

Sharding: core c handles batch b=c//4 and heads 4*(c%4)..4*(c%4)+3.
Each core computes qkv/gate projections for its heads from x[b], per-head
QK-RMS-norm + softmax attention + sigmoid gating, and a row-split o_proj
partial [S, D] (bf16). Host sums the 4 partials per batch + residual.

Key structure:
- All big matmuls run in fp8e4m3 with DoubleRow (contraction pairs packed
  into the free dim of both operands: [Ki, 2, M] x [Ki, 2, N]). Weights are
  pre-scaled x32 on host so fp8 sees ~unit-variance values; descales fold
  into copyback scales.
- attnV emits [sq, hd] tiles (out free = 65) instead of [hd, sq]: softmax
  sums + sigmoid gate become per-partition scalars. A per-sq-tile PE
  transpose rebuilds the [hd, sq] layout o_proj needs, writing into the
  just-drained po PSUM slot.
- exp is the elementwise wall (16.8M elems/core): batched to N=1024 per
  instruction and split between ACT (native Exp) and a custom single-pass
  8-stage DVE op computing exp(t) ~= (1 + t/64)^64.
- softmax runs without max-subtraction; a uniform -3*ln2 bias keeps exp
  outputs under the fp8e4m3 max (e^(8-2.079)=372<448; |s|<=8 by
  Cauchy-Schwarz after QK RMS norm). The bias cancels in normalization.
- v carries an all-ones column so attnV also yields the softmax sums.
"""

import json
import math

import numpy as np
import ml_dtypes

import concourse.bass as bass
import concourse.bass_utils as bass_utils
import concourse.bass2jax as bass2jax
import concourse.mybir as mybir
import concourse.tile as tile
import concourse.dve_ops as dve_ops
from concourse.dve_ops import DveOp
from concourse.dve_spec import Spec, Src0, C0, C1, sq as dve_sq
from concourse.tile import TileContext
from concourse.masks import make_identity
from concourse.vector_clock import ScopedClock, VectorClock

F32 = mybir.dt.float32
BF16 = mybir.dt.bfloat16
F8 = mybir.dt.float8e4
AF = mybir.ActivationFunctionType
ALU = mybir.AluOpType
DR = mybir.MatmulPerfMode.DoubleRow
BF = ml_dtypes.bfloat16
F8NP = mybir.dt.np(mybir.dt.float8e4)

B, S, D = 2, 2048, 1024
NH_TOT, HD = 16, 64
NH = 4            # heads per core
EPS = 1e-5
P = 128
ST = S // P       # 16 s-tiles
KTP = 4           # d-dim pair-tiles (4 x (2x128))
EXPB = 3 * math.log(2.0)          # uniform score bias (cancels in softmax)
C1EXP = 1.0 - EXPB / 64.0         # dve exp: a = s*rk/64 + C1EXP

# engine split knobs (tuned against sim engine-busy readout)
# NOTE: this container's walrus cannot codegen custom-DVE ops ("ISA wrong
# length" even for the production RECIPROCAL_APPROX_FAST), so exp runs
# entirely on ACT and everything else moves to DVE.
EXP_ACT_OF_32 = 32   # of the 32 exp half-tiles per head, this many on ACT

# ----------------------------------------------------------------------------
# custom DVE op: exp(t) ~= (1 + t/64)^64, one pass, 8 uop stages
# ----------------------------------------------------------------------------


def _ref_exp64(in0, in1, s0, s1, imm2):
    a = in0.astype(np.float32) * np.asarray(s0, np.float32) + np.float32(s1)
    for _ in range(6):
        a = a * a
    return a


EXP64_ANT = DveOp(
    "EXP64_ANT",
    Spec(
        body=dve_sq(dve_sq(dve_sq(dve_sq(dve_sq(dve_sq(Src0 * C0 + C1)))))),
        reference=_ref_exp64,
    ),
    subdim=False,
    uops_sha={"v3": "8299cc4e9a89acf1", "v4": "df7b3d1456faeb1a"},
)


def _register_exp_op():
    if EXP64_ANT.name in dve_ops.CUSTOM_DVE_SPECS:
        return
    row = max(dve_ops._SUB_OPCODE_FOR_NAME.values()) + 1
    assert row < 0x20
    dve_ops.OPS.append(EXP64_ANT)
    dve_ops.CUSTOM_DVE_SPECS[EXP64_ANT.name] = EXP64_ANT.spec
    dve_ops._SUB_OPCODE_FOR_NAME[EXP64_ANT.name] = row


_register_exp_op()

# ----------------------------------------------------------------------------
# compat patches: this walrus build accepts only ONE sync-wait per instruction
# ----------------------------------------------------------------------------

def _patched_drain_and_barrier(self, tick_clock, wait_clock):
    nc = self.nc
    gc = tick_clock.global_clock
    n = len(gc)
    for p in range(n):
        t = gc[p]
        if t <= 0:
            continue
        vec = VectorClock([0] * n)
        vec.require_at_least(p, t)
        nop = nc.sync.nop(nofuse=True, hint=f"drain_wait_p{p}")
        wait_clock.add_sem_waits(nop.ins, ScopedClock({None: vec}))
    nc.sync.drain(fusable=False)
    nc.all_engine_barrier()
    assert self.sems is not None
    popped = nc._tile_sem_poison_stack.pop()
    assert popped is self._sem_poison
    nc.clear_and_free_semaphores(list(self.sems.allocated().values()))
    nc.all_engine_barrier()


def _split_multi_waits(bir_json: bytes) -> bytes:
    bj = json.loads(bir_json)
    n_split = 0
    for fn in bj.get("functions", []):
        for blk in fn.get("blocks", []):
            out = []
            for inst in blk.get("instructions", []):
                si = inst.get("sync_info")
                waits = si.get("on_wait", []) if si else []
                if len(waits) > 1:
                    for i, w in enumerate(waits[:-1]):
                        out.append({
                            "debug": inst.get("debug"),
                            "engine": inst["engine"],
                            "ins": [], "outs": [],
                            "name": f"{inst['name']}_sw{i}",
                            "opcode": "NoOp",
                            "sync_info": {"on_update": [], "on_wait": [w]},
                            "text_hint": "split_wait",
                        })
                        n_split += 1
                    si["on_wait"] = [waits[-1]]
                out.append(inst)
            blk["instructions"] = out
    if n_split:
        return json.dumps(bj).encode()
    return bir_json


_ORIG_COMPILE = bass_utils.compile_bir_kernel


def _patched_compile_bir_kernel(bir_json, tmpdir, neff_name="file.neff"):
    return _ORIG_COMPILE(_split_multi_waits(bir_json), tmpdir, neff_name)


def _apply_compat():
    tile.TileContext._drain_and_barrier = _patched_drain_and_barrier
    bass_utils.compile_bir_kernel = _patched_compile_bir_kernel
    bass2jax.compile_bir_kernel = _patched_compile_bir_kernel


_apply_compat()

# ----------------------------------------------------------------------------
# device program (SPMD: identical program, per-core data)
# ----------------------------------------------------------------------------

_NC_CACHE = None


def _build_program():
    nc = bass.Bass()
    xt8 = nc.declare_dram_parameter("xt8", [P, KTP, 2, S], F8, isOutput=False)
    wqk8 = nc.declare_dram_parameter("wqk8", [P, KTP, 2, 512], F8, isOutput=False)
    wvg8 = nc.declare_dram_parameter("wvg8", [P, KTP, 2, 272], F8, isOutput=False)
    wo8 = nc.declare_dram_parameter("wo8", [64, NH, D], F8, isOutput=False)
    rv = nc.declare_dram_parameter("rv", [P, ST], F32, isOutput=False)
    rv4 = nc.declare_dram_parameter("rv4", [P, 64], F32, isOutput=False)
    qn = nc.declare_dram_parameter("qn", [P, 1], F32, isOutput=False)
    kn = nc.declare_dram_parameter("kn", [P, 1], F32, isOutput=False)
    ind = nc.declare_dram_parameter("ind", [P, 2], BF16, isOutput=False)
    ind2 = nc.declare_dram_parameter("ind2", [2, P], BF16, isOutput=False)
    outp = nc.declare_dram_parameter("out_p", [S, D], BF16, isOutput=True)

    with TileContext(nc) as tc:
        with tc.tile_pool(name="big", bufs=1) as big, \
             tc.tile_pool(name="work", bufs=4) as work, \
             tc.tile_pool(name="wex", bufs=9) as wex, \
             tc.tile_pool(name="wsb", bufs=4) as wsb, \
             tc.tile_pool(name="wot", bufs=4) as wot, \
             tc.tile_pool(name="psc", bufs=2, space="PSUM") as psc, \
             tc.tile_pool(name="ppo", bufs=1, space="PSUM") as ppo:

            # ---- resident inputs (weights first; x chunked by s-range so
            # the first projection iterations start early; wo8 needed last)
            wqks = big.tile([P, KTP, 2, 512], F8)
            nc.sync.dma_start(out=wqks[:], in_=wqk8[:, :, :, :])
            xts = big.tile([P, KTP, 2, S], F8)
            for xc in range(4):
                nc.sync.dma_start(
                    out=xts[:, :, :, 512 * xc:512 * xc + 512],
                    in_=xt8[:, :, :, 512 * xc:512 * xc + 512])
            wvgs = big.tile([P, KTP, 2, 272], F8)
            nc.sync.dma_start(out=wvgs[:], in_=wvg8[:, :, :, :])
            wos = big.tile([64, NH, D], F8)
            nc.gpsimd.dma_start(out=wos[:], in_=wo8[:, :, :])
            # tiny scale/selector params ride the gpsimd DMA queue so they
            # land immediately instead of behind the big weight DMAs on SP
            qns = big.tile([P, 1], F32)
            nc.gpsimd.dma_start(out=qns[:], in_=qn[:, :])
            kns = big.tile([P, 1], F32)
            nc.gpsimd.dma_start(out=kns[:], in_=kn[:, :])
            inds = big.tile([P, 2], BF16)
            nc.gpsimd.dma_start(out=inds[:], in_=ind[:, :])
            ind2s = big.tile([2, P], BF16)
            nc.gpsimd.dma_start(out=ind2s[:], in_=ind2[:, :])
            rvs = big.tile([P, ST], F32)
            nc.gpsimd.dma_start(out=rvs[:], in_=rv[:, :])
            rv4s = big.tile([P, 64], F32)
            nc.gpsimd.dma_start(out=rv4s[:], in_=rv4[:, :])

            ident = big.tile([P, P], F32)
            make_identity(nc, ident[:])
            epsb = big.tile([P, 1], F32)
            nc.vector.memset(epsb[:], EPS)
            eps64 = big.tile([P, 1], F32)
            nc.vector.memset(eps64[:], HD * EPS)
            expbb = big.tile([P, 1], F32)
            nc.vector.memset(expbb[:], -EXPB)

            # ---- resident intermediates
            qkT = big.tile([P, 4, S], BF16)          # mt0,1=q(pair0,1) mt2,3=k
            vbuf = big.tile([P, ST // 2, 2, NH, 68], F8)  # [sk%128,sktp,e,j,hd+1]
            nc.vector.memset(vbuf[:, :, :, :, 64:65], 1.0)
            obuf = big.tile([64, NH, S], F8)         # gated attn outT per head
            gnat = big.tile([P, 16, 4], F32)         # tanh(r*gate/2) [t, j]
            gnat3 = big.tile([P, 16, 4], F32)        # 16*(1+tanh) = 32*sigmoid
            gstage = big.tile([P, 16, 4], F32)       # raw gate rows
            rkb = big.tile([P, 64], F32)             # 1/rms(k), col=4skt+j
            rkb64 = big.tile([P, 64], F32)           # rkb/64 (dve exp scale)
            ssum = big.tile([2, 2, S], F32)          # q sumsq rows per mt
            rqb = big.tile([2, 2, S], BF16)          # q scale rows per mt
            tmpq = big.tile([2, 2, S], F32)
            tmpk = big.tile([P, 64], F32)
            graw = big.tile([P, 64], F32)
            rs = big.tile([P, 16], F32)              # recip softmax sums
            crb = big.tile([P, 16], F32)             # 32*gate*rs per sq-tile

            # po layout: [bank(4), slot(4), col(128)]; slice q -> [q//4, q%4]
            # pre-attention scratch carved from the same banks:
            #   gate psum = pot[:, 0, 0, 0:64]; k-sumsq psum = pot[:, 0, 1, 0:64]
            pot = ppo.tile([P, 4, 512], F32, tag="po")

            # k-norm column layout: kcol(skt, j) = 32*(j//2) + 2*skt + (j%2)
            # so each k head-pair's stats occupy one contiguous 32-col half.
            def emit_sums(mt, ch, sqt):
                if mt < 2:  # q: row-layout sums [2, 512]
                    prf = proj_psum()
                    nc.tensor.matmul(prf(512)[0:2], inds[:], sqt[:],
                                     start=True, stop=True)
                    nc.vector.tensor_copy(
                        out=ssum[:, mt, 512 * ch:512 * ch + 512],
                        in_=prf(512)[0:2])
                else:  # k: column sums
                    for sl in range(4):
                        skt = ch * 4 + sl
                        c0 = 128 + 32 * (mt - 2) + 2 * skt
                        nc.tensor.matmul(
                            pot[:, 0, c0:c0 + 2],
                            sqt[:, 128 * sl:128 * sl + 128], inds[:],
                            start=True, stop=True)

            # projection iteration; elementwise on ACT only where allowed
            # (anything on the ACT FIFO ahead of the exps delays attention).
            # Projection psum rotates over 4 slots: the 2-deep "sc" ring plus
            # pot banks 2/3, which attention doesn't touch until its q>=8
            # attnV writes (far later, ordered by the tile framework).
            pend = []
            pslot = [0]

            def proj_psum():
                s = pslot[0] = 2 + (pslot[0] + 1) % 2
                return lambda n, s=s: pot[:, s, 0:n]

            def emit_proj(mt, ch, act_ok):
                if act_ok:
                    pqt = psc.tile([P, 1024], F32, tag="sc", name="pqt")
                    pqf = lambda n: pqt[:, 0:n]
                else:
                    pqf = proj_psum()
                for ktp in range(KTP):
                    nc.tensor.matmul(
                        pqf(512),
                        wqks[:, ktp, :, 128 * mt:128 * mt + 128],
                        xts[:, ktp, :, 512 * ch:512 * ch + 512],
                        start=(ktp == 0), stop=(ktp == KTP - 1),
                        perf_mode=DR)
                sc = qns if mt < 2 else kns
                qsl = qkT[:, mt, 512 * ch:512 * ch + 512]
                if act_ok:
                    nc.scalar.activation(qsl, pqf(512), AF.Copy,
                                         scale=sc[:])
                else:
                    nc.vector.tensor_scalar(qsl, pqf(512), sc[:],
                                            None, op0=ALU.mult)
                # squares from the bf16 copy (sbuf 2-byte: fast DVE path)
                sqt = work.tile([P, 512], BF16, tag="sq")
                nc.vector.tensor_tensor(sqt[:], qsl, qsl, ALU.mult)
                pend.append((mt, ch, sqt))
                if len(pend) > 2:
                    emit_sums(*pend.pop(0))

            def emit_khalf(half):
                sl = slice(32 * half, 32 * half + 32)
                psl = slice(128 + 32 * half, 160 + 32 * half)
                nc.scalar.activation(tmpk[:, sl], pot[:, 0, psl],
                                     AF.Sqrt, bias=epsb[:], scale=1.0 / HD)
                nc.vector.reciprocal(rkb[:, sl], tmpk[:, sl])
                nc.vector.tensor_scalar(rkb64[:, sl], rkb[:, sl],
                                        1.0 / 64.0, None, op0=ALU.mult)

            def emit_qscale(mt, late=False):
                for chq in range(4):
                    if late:
                        pbct = psc.tile([P, 1024], F32, tag="sc", name="pbct")
                        pbc = lambda n: pbct[:, 0:n]
                    else:
                        pbc = proj_psum()
                    nc.tensor.matmul(
                        pbc(512), ind2s[:],
                        rqb[:, mt, 512 * chq:512 * chq + 512],
                        start=True, stop=True)
                    nc.vector.tensor_tensor(
                        qkT[:, mt, 512 * chq:512 * chq + 512],
                        qkT[:, mt, 512 * chq:512 * chq + 512],
                        pbc(512), ALU.mult)

            # ---- phase C part 1: mt0 (q heads 0/1) + mt2 (k heads 0/1),
            # interleaved by s-chunk to match the x DMA chunk arrival
            for mt, ch in [(m, c) for c in range(4) for m in (0, 2)]:
                emit_proj(mt, ch, act_ok=True)
            while pend:
                emit_sums(*pend.pop(0))
            # rq(mt0) in s-halves: the first scores need only half 0, so
            # the exp stream starts ~5us earlier
            for h2 in range(2):
                sl = slice(1024 * h2, 1024 * h2 + 1024)
                nc.scalar.activation(tmpq[:, 0, sl], ssum[:, 0, sl],
                                     AF.Sqrt, bias=eps64[0:2, :], scale=1.0)
                with nc.allow_low_precision(reason="rq in bf16 for mm"):
                    nc.vector.reciprocal(rqb[:, 0, sl], tmpq[:, 0, sl])
                for chq in (2 * h2, 2 * h2 + 1):
                    pbc = proj_psum()
                    nc.tensor.matmul(
                        pbc(512), ind2s[:],
                        rqb[:, 0, 512 * chq:512 * chq + 512],
                        start=True, stop=True)
                    nc.vector.tensor_tensor(
                        qkT[:, 0, 512 * chq:512 * chq + 512],
                        qkT[:, 0, 512 * chq:512 * chq + 512],
                        pbc(512), ALU.mult)
            emit_khalf(0)

            # ---- phase E helper: v + gate projection (fp8 DoubleRow).
            # Emitted inside attention cycle 0, hidden under head-0's exps.
            def emit_E(t):
                pvf = proj_psum()
                for ktp in range(KTP):
                    nc.tensor.matmul(pvf(260),
                                     xts[:, ktp, :, 128 * t:128 * t + 128],
                                     wvgs[:, ktp, :, 0:260],
                                     start=(ktp == 0), stop=(ktp == KTP - 1),
                                     perf_mode=DR)
                vdst = vbuf[:, t // 2, t % 2, :, 0:64]
                nc.vector.tensor_scalar(vdst, pvf(260)[0:P, 0:256],
                                        rvs[:, t:t + 1], None, op0=ALU.mult)
                nc.vector.tensor_copy(out=gstage[:, t, :],
                                      in_=pvf(260)[0:P, 256:260])

            # ---- phase C part 2: mt3 (k heads 2/3) + mt1 (q heads 2/3).
            # All elementwise goes to the DVE: it drains during head-0/1
            # attention while the ACT is saturated with exp. The norm chains
            # for these heads are emitted inside the attention pipeline.
            for mt, ch in [(m, c) for m in (3, 1) for c in range(4)]:
                emit_proj(mt, ch, act_ok=False)
            while pend:
                emit_sums(*pend.pop(0))
            emit_khalf(1)
            nc.scalar.activation(tmpq[:, 1, :], ssum[:, 1, :], AF.Sqrt,
                                 bias=eps64[0:2, :], scale=1.0)
            with nc.allow_low_precision(reason="rq in bf16 for mm"):
                nc.vector.reciprocal(rqb[:, 1, :], tmpq[:, 1, :])
            emit_qscale(1)

            # ---- attention, software-pipelined over heads:
            # cycle jc: scores+exp for head jc interleaved with the finalize
            # of head jc-1 (gated copy, transpose into drained po slot,
            # 4-batched copyback to obuf); attnV for head jc runs as one
            # block at the end (ex tiles buffered in a deep ring), after all
            # of head jc-1's transposes, so the po banks swap owners cleanly.
            # DVE exp ladder: exp(t) ~= (1+t/64)^64 via 6 squarings (f32 for
            # the first three, bf16 after; the DVE runs 2-byte sbuf ops at
            # 2-4x). ~6x the ACT cost per tile, but it spends otherwise-idle
            # DVE cycles to shave the ACT-bound attention phase.
            def emit_exp_dve(ps, dst, c):
                e0 = work.tile([P, 1024], F32, tag="e0")
                nc.vector.tensor_scalar(e0[:], ps[:], rkb64[:, c:c + 1],
                                        C1EXP, op0=ALU.mult, op1=ALU.add)
                nc.vector.tensor_tensor(e0[:], e0[:], e0[:], ALU.mult)
                nc.vector.tensor_tensor(e0[:], e0[:], e0[:], ALU.mult)
                e1 = work.tile([P, 1024], BF16, tag="e1")
                with nc.allow_low_precision(reason="softmax wts are fp8"):
                    nc.vector.tensor_tensor(e1[:], e0[:], e0[:], ALU.mult)
                    nc.vector.tensor_tensor(e1[:], e1[:], e1[:], ALU.mult)
                    nc.vector.tensor_tensor(e1[:], e1[:], e1[:], ALU.mult)
                    nc.vector.tensor_tensor(dst, e1[:], e1[:], ALU.mult)

            exts = {}
            # attnV emission schedule: (sktp, q) lands at the first loop
            # index where its ex pair is computed AND po slot q's batch has
            # been copied back to obuf for the previous head (tp of slot q
            # precedes it in the PE FIFO, so no cross-engine deadlock).
            avsched = {}
            for sktp_ in range(8):
                for q_ in range(16):
                    avsched.setdefault(
                        max(2 * sktp_ + 1, 4 * (q_ // 4) + 3), []).append(
                            (sktp_, q_))
            for v_ in avsched.values():
                v_.sort()
            # cycle 0 variant: pot banks 2/3 double as phase-E psum slots, so
            # the q>=8 attnV (which overwrites them) waits for the last E tile
            avsched0 = {}
            for sktp_ in range(8):
                for q_ in range(16):
                    avsched0.setdefault(
                        15 if q_ >= 8 else max(2 * sktp_ + 1,
                                               4 * (q_ // 4) + 3), []).append(
                            (sktp_, q_))
            for v_ in avsched0.values():
                v_.sort()
            def emit_scores_exp(j, skt, h):
                mtq, mtk = j // 2, 2 + j // 2
                a = 64 * (j % 2)
                if (j, skt // 2) not in exts:
                    exts[(j, skt // 2)] = wex.tile(
                        [P, 2, S], F8, tag="ex", name="ext")
                ext = exts[(j, skt // 2)]
                ps = psc.tile([P, 1024], F32, tag="sc")
                for c2 in range(2):
                    q0 = 1024 * h + 512 * c2
                    nc.tensor.matmul(
                        ps[:, 512 * c2:512 * c2 + 512],
                        qkT[a:a + 64, mtk, 128 * skt:128 * skt + 128],
                        qkT[a:a + 64, mtq, q0:q0 + 512],
                        start=True, stop=True)
                col = 32 * (j // 2) + 2 * skt + (j % 2)
                nc.scalar.activation(
                    ext[:, skt % 2, 1024 * h:1024 * h + 1024], ps[:],
                    AF.Exp, bias=expbb[:], scale=rkb[:, col:col + 1])

            def emit_fin(jf, q, act_mix):
                posb = wsb.tile([P, 64], F32, tag="posb")
                qo = 128 * (q % 4)
                src = pot[:, q // 4, qo:qo + 64]
                if act_mix and q % 2 == 0:
                    nc.scalar.activation(posb[:], src, AF.Copy,
                                         scale=crb[:, q:q + 1])
                else:
                    nc.vector.tensor_scalar(posb[:], src, crb[:, q:q + 1],
                                            None, op0=ALU.mult)
                # transpose into the just-drained po slot
                nc.tensor.transpose(pot[0:64, q // 4, qo:qo + 128],
                                    posb[:], ident[:])
                if q % 4 == 3:
                    b = q // 4
                    dst = obuf[:, jf, 512 * b:512 * b + 512]
                    srcq = pot[0:64, b, 0:512]
                    if act_mix and b % 2 == 1:
                        nc.scalar.activation(dst, srcq, AF.Copy)
                    else:
                        nc.vector.tensor_copy(out=dst, in_=srcq)

            def emit_attnv(j, sktp, q):
                nc.tensor.matmul(
                    pot[:, q // 4, 128 * (q % 4):128 * (q % 4) + 65],
                    exts[(j, sktp)][:, :, 128 * q:128 * q + 128],
                    vbuf[:, sktp, :, j, 0:65],
                    start=(sktp == 0), stop=(sktp == 7), perf_mode=DR)

            def emit_oproj(t):
                ot = wot.tile([P, 1024], BF16, tag="ot")
                pp = psc.tile([P, 1024], F32, tag="sc")
                for nh in range(2):
                    for jp in range(2):
                        nc.tensor.matmul(
                            pp[:, 512 * nh:512 * nh + 512],
                            obuf[:, 2 * jp:2 * jp + 2,
                                 128 * t:128 * t + 128],
                            wos[:, 2 * jp:2 * jp + 2,
                                512 * nh:512 * nh + 512],
                            start=(jp == 0), stop=(jp == 1), perf_mode=DR)
                if t % 2 == 0:
                    nc.scalar.activation(ot[:], pp[:], AF.Copy,
                                         scale=2.0 ** -10)
                else:
                    nc.vector.tensor_scalar(ot[:], pp[:], 2.0 ** -10,
                                            None, op0=ALU.mult)
                if t % 2 == 0:
                    nc.sync.dma_start(out=outp[128 * t:128 * t + 128, :],
                                      in_=ot[:])
                else:
                    nc.gpsimd.dma_start(out=outp[128 * t:128 * t + 128, :],
                                        in_=ot[:])

            # heads 0-2: scores+exp of head j over the finalize of head j-1
            for jc in range(3):
                j = jc
                jf = jc - 1 if jc > 0 else None
                if jc == 1:
                    # gate: 32*sigmoid(rg) = 16*(1+tanh(rg/2)); Tanh shares
                    # the exp table set, so no ACT table reload here
                    nc.vector.tensor_tensor(graw[:], gstage[:], rv4s[:],
                                            ALU.mult)
                    nc.scalar.activation(gnat[:], graw[:], AF.Tanh,
                                         scale=0.5)
                    nc.vector.tensor_scalar(gnat3[:], gnat[:], 1.0, 16.0,
                                            op0=ALU.add, op1=ALU.mult)
                if jf is not None:
                    nc.vector.reciprocal(rs[:], pot[:, :, 64:512:128])
                    nc.vector.tensor_tensor(crb[:], rs[:], gnat3[:, :, jf],
                                            ALU.mult)
                for skt in range(ST):
                    emit_scores_exp(j, skt, 0)
                    emit_scores_exp(j, skt, 1)
                    if jf is not None:
                        emit_fin(jf, skt, False)
                    if jc == 0:
                        emit_E(skt)
                    sched = avsched0 if jc == 0 else avsched
                    for sktp, q in sched.get(skt, []):
                        emit_attnv(j, sktp, q)

            # head 3, pass A: h=0 exps cover attnV for sq-tiles 0-7;
            # head-2 finalize interleaves as usual
            nc.vector.reciprocal(rs[:], pot[:, :, 64:512:128])
            nc.vector.tensor_tensor(crb[:], rs[:], gnat3[:, :, 2],
                                    ALU.mult)
            for skt in range(ST):
                emit_scores_exp(3, skt, 0)
                emit_fin(2, skt, False)
                for sktp, q in avsched.get(skt, []):
                    if q < 8:
                        emit_attnv(3, sktp, q)
            # pass B: h=1 exps; head-3's low sq-tiles finalize and the first
            # half of o_proj runs underneath them
            nc.vector.reciprocal(rs[:, 0:8], pot[:, 0:2, 64:512:128])
            nc.vector.tensor_tensor(crb[:, 0:8], rs[:, 0:8],
                                    gnat3[:, 0:8, 3], ALU.mult)
            for idx in range(ST):
                emit_scores_exp(3, idx, 1)
                if idx % 2 == 0:
                    emit_fin(3, idx // 2, False)
                else:
                    for q in range(8, 16):
                        emit_attnv(3, (idx - 1) // 2, q)
                if idx >= 7 and idx % 2 == 1:
                    emit_oproj((idx - 7) // 2)
            # pass C: high sq-tiles of head 3 + the rest of o_proj
            nc.vector.reciprocal(rs[:, 8:16], pot[:, 2:4, 64:512:128])
            nc.vector.tensor_tensor(crb[:, 8:16], rs[:, 8:16],
                                    gnat3[:, 8:16, 3], ALU.mult)
            oq = [4], [5], [6], [7, 8], [9], [10], [11], [12, 13, 14, 15]
            for i, q in enumerate(range(8, 16)):
                emit_fin(3, q, True)
                for t in oq[i]:
                    emit_oproj(t)
    return nc


def _get_program():
    global _NC_CACHE
    if _NC_CACHE is None:
        _NC_CACHE = _build_program()
    return _NC_CACHE


# ----------------------------------------------------------------------------
# host wrapper
# ----------------------------------------------------------------------------

def _prep_inputs(x, prenorm_w, qkv_w, gate_w, o_w, q_norm_w, k_norm_w):
    x = np.asarray(x, np.float32)
    pw = np.asarray(prenorm_w, np.float32)
    qkv_w = np.asarray(qkv_w, np.float32)
    gate_w = np.asarray(gate_w, np.float32)
    o_w = np.asarray(o_w, np.float32)
    qw = qkv_w[0:D] * pw[None, :]
    kw = qkv_w[D:2 * D] * pw[None, :]
    vw = qkv_w[2 * D:3 * D] * pw[None, :]
    gw = gate_w * pw[None, :]

    r = 1.0 / np.sqrt(np.mean(x * x, axis=-1) + EPS)      # [B, S]
    ind = np.zeros((P, 2), BF)
    ind[0:64, 0] = 1
    ind[64:128, 1] = 1
    ind2 = np.zeros((2, P), BF)
    ind2[0, 0:64] = 1
    ind2[1, 64:128] = 1
    qn = (np.tile(np.asarray(q_norm_w, np.float32), 2) / 32.0)[:, None]
    kn = (np.tile(np.asarray(k_norm_w, np.float32), 2) / 32.0)[:, None]

    in_maps = []
    for c in range(8):
        b, hg = c // 4, c % 4
        hsl = slice(256 * hg, 256 * hg + 256)
        xtc = np.ascontiguousarray(
            x[b].T.reshape(KTP, 2, P, S).transpose(2, 0, 1, 3)).astype(F8NP)
        wqk = np.concatenate([qw[hsl], kw[hsl]], 0).T * 32.0  # [1024, 512]
        wqkc = np.ascontiguousarray(
            wqk.reshape(KTP, 2, P, 512).transpose(2, 0, 1, 3)).astype(F8NP)
        wvg = np.concatenate([vw[hsl], gw[4 * hg:4 * hg + 4]], 0).T * 32.0
        wvgp = np.zeros((D, 272), np.float32)
        wvgp[:, 0:260] = wvg
        wvgc = np.ascontiguousarray(
            wvgp.reshape(KTP, 2, P, 272).transpose(2, 0, 1, 3)).astype(F8NP)
        wo = o_w[:, hsl].T.reshape(NH, 64, D).transpose(1, 0, 2) * 32.0
        woc = np.ascontiguousarray(wo).astype(F8NP)
        rvc = np.ascontiguousarray(
            r[b].reshape(ST, P).T / 32.0).astype(np.float32)
        rv4c = np.ascontiguousarray(np.repeat(rvc, 4, axis=1))
        in_maps.append({
            "xt8": xtc, "wqk8": wqkc, "wvg8": wvgc, "wo8": woc,
            "rv": rvc, "rv4": rv4c, "qn": qn.astype(np.float32),
            "kn": kn.astype(np.float32), "ind": ind, "ind2": ind2,
        })
    return in_maps


_RUNNER = None


def _get_runner():
    """Build the sharded PJRT executable ONCE and reuse it across calls
    (run_bass_kernel_spmd re-traces/re-compiles on every invocation)."""
    global _RUNNER
    if _RUNNER is not None:
        return _RUNNER
    import jax
    import concourse.mybir as _mybir
    from concourse.bass2jax import (_bass_exec_p, partition_id_tensor,
                                    install_neuronx_cc_hook, Mesh,
                                    PartitionSpec, shard_map)
    install_neuronx_cc_hook()
    nc = _get_program()
    n_cores = 8
    partition_name = (nc.partition_id_tensor.name
                      if nc.partition_id_tensor else None)
    in_names, out_names, out_avals, zero_outs = [], [], [], []
    for alloc in nc.m.functions[0].allocations:
        if not isinstance(alloc, _mybir.MemoryLocationSet):
            continue
        name = alloc.memorylocations[0].name
        if alloc.kind == "ExternalInput":
            if name != partition_name:
                in_names.append(name)
        elif alloc.kind == "ExternalOutput":
            shape = tuple(alloc.tensor_shape)
            dtype = _mybir.dt.np(alloc.dtype)
            out_names.append(name)
            out_avals.append(jax.core.ShapedArray(shape, dtype))
            zero_outs.append(np.zeros(shape, dtype))
    n_params = len(in_names)
    n_outs = len(out_avals)
    all_in = list(in_names) + list(out_names)
    if partition_name is not None:
        all_in.append(partition_name)
    donate = tuple(range(n_params, n_params + n_outs))

    def _body(*args):
        operands = list(args)
        if partition_name is not None:
            operands.append(partition_id_tensor())
        return tuple(_bass_exec_p.bind(
            *operands, out_avals=tuple(out_avals), in_names=tuple(all_in),
            out_names=tuple(out_names), lowering_input_output_aliases=(),
            sim_require_finite=True, sim_require_nnan=True, nc=nc))

    devices = jax.devices()[:n_cores]
    mesh = Mesh(np.asarray(devices), ("core",))
    sharded = jax.jit(
        shard_map(_body, mesh=mesh,
                  in_specs=(PartitionSpec("core"),) * (n_params + n_outs),
                  out_specs=(PartitionSpec("core"),) * n_outs,
                  check_rep=False),
        donate_argnums=donate, keep_unused=True)
    _RUNNER = (sharded, in_names, out_names, out_avals, zero_outs, n_cores)
    return _RUNNER


def kernel(x, prenorm_w, qkv_w, gate_w, o_w, q_norm_w, k_norm_w):
    sharded, in_names, out_names, out_avals, zero_outs, n_cores = _get_runner()
    in_maps = _prep_inputs(x, prenorm_w, qkv_w, gate_w, o_w,
                           q_norm_w, k_norm_w)
    concat_in = [np.concatenate([in_maps[c][nm] for c in range(n_cores)], 0)
                 for nm in in_names]
    concat_zeros = [np.zeros((n_cores * z.shape[0], *z.shape[1:]), z.dtype)
                    for z in zero_outs]
    out_arrs = sharded(*concat_in, *concat_zeros)
    oi = out_names.index("out_p")
    op = np.asarray(out_arrs[oi]).astype(np.float32).reshape(
        n_cores, *out_avals[oi].shape)
    outs = [op[c] for c in range(n_cores)]
    x = np.asarray(x, np.float32)
    y0 = x[0] + outs[0] + outs[1] + outs[2] + outs[3]
    y1 = x[1] + outs[4] + outs[5] + outs[6] + outs[7]
    return np.stack([y0, y1]).astype(np.float32)


# revision 55
# speedup vs baseline: 1.6956x; 1.0201x over previous
"""GatedAttention Trainium2 kernel (8 NeuronCores, tensor-parallel over (batch, head-group)).

Sharding: core c handles batch b=c//4 and heads 4*(c%4)..4*(c%4)+3.
Each core computes qkv/gate projections for its heads from x[b], per-head
QK-RMS-norm + softmax attention + sigmoid gating, and a row-split o_proj
partial [S, D] (bf16). Host sums the 4 partials per batch + residual.

Key structure:
- All big matmuls run in fp8e4m3 with DoubleRow (contraction pairs packed
  into the free dim of both operands: [Ki, 2, M] x [Ki, 2, N]). Weights are
  pre-scaled x32 on host so fp8 sees ~unit-variance values; descales fold
  into copyback scales.
- attnV emits [sq, hd] tiles (out free = 65) instead of [hd, sq]: softmax
  sums + sigmoid gate become per-partition scalars. A per-sq-tile PE
  transpose rebuilds the [hd, sq] layout o_proj needs, writing into the
  just-drained po PSUM slot.
- exp is the elementwise wall (16.8M elems/core): batched to N=1024 per
  instruction and split between ACT (native Exp) and a custom single-pass
  8-stage DVE op computing exp(t) ~= (1 + t/64)^64.
- softmax runs without max-subtraction; a uniform -3*ln2 bias keeps exp
  outputs under the fp8e4m3 max (e^(8-2.079)=372<448; |s|<=8 by
  Cauchy-Schwarz after QK RMS norm). The bias cancels in normalization.
- v carries an all-ones column so attnV also yields the softmax sums.
"""

import json
import math

import numpy as np
import ml_dtypes

import concourse.bass as bass
import concourse.bass_utils as bass_utils
import concourse.bass2jax as bass2jax
import concourse.mybir as mybir
import concourse.tile as tile
import concourse.dve_ops as dve_ops
from concourse.dve_ops import DveOp
from concourse.dve_spec import Spec, Src0, C0, C1, sq as dve_sq
from concourse.tile import TileContext
from concourse.masks import make_identity
from concourse.vector_clock import ScopedClock, VectorClock

F32 = mybir.dt.float32
BF16 = mybir.dt.bfloat16
F8 = mybir.dt.float8e4
AF = mybir.ActivationFunctionType
ALU = mybir.AluOpType
DR = mybir.MatmulPerfMode.DoubleRow
BF = ml_dtypes.bfloat16
F8NP = mybir.dt.np(mybir.dt.float8e4)

B, S, D = 2, 2048, 1024
NH_TOT, HD = 16, 64
NH = 4            # heads per core
EPS = 1e-5
P = 128
ST = S // P       # 16 s-tiles
KTP = 4           # d-dim pair-tiles (4 x (2x128))
EXPB = 3 * math.log(2.0)          # uniform score bias (cancels in softmax)
C1EXP = 1.0 - EXPB / 64.0         # dve exp: a = s*rk/64 + C1EXP

# engine split knobs (tuned against sim engine-busy readout)
# NOTE: this container's walrus cannot codegen custom-DVE ops ("ISA wrong
# length" even for the production RECIPROCAL_APPROX_FAST), so exp runs
# entirely on ACT and everything else moves to DVE.
EXP_ACT_OF_32 = 32   # of the 32 exp half-tiles per head, this many on ACT

# ----------------------------------------------------------------------------
# custom DVE op: exp(t) ~= (1 + t/64)^64, one pass, 8 uop stages
# ----------------------------------------------------------------------------


def _ref_exp64(in0, in1, s0, s1, imm2):
    a = in0.astype(np.float32) * np.asarray(s0, np.float32) + np.float32(s1)
    for _ in range(6):
        a = a * a
    return a


EXP64_ANT = DveOp(
    "EXP64_ANT",
    Spec(
        body=dve_sq(dve_sq(dve_sq(dve_sq(dve_sq(dve_sq(Src0 * C0 + C1)))))),
        reference=_ref_exp64,
    ),
    subdim=False,
    uops_sha={"v3": "8299cc4e9a89acf1", "v4": "df7b3d1456faeb1a"},
)


def _register_exp_op():
    if EXP64_ANT.name in dve_ops.CUSTOM_DVE_SPECS:
        return
    row = max(dve_ops._SUB_OPCODE_FOR_NAME.values()) + 1
    assert row < 0x20
    dve_ops.OPS.append(EXP64_ANT)
    dve_ops.CUSTOM_DVE_SPECS[EXP64_ANT.name] = EXP64_ANT.spec
    dve_ops._SUB_OPCODE_FOR_NAME[EXP64_ANT.name] = row


_register_exp_op()

# ----------------------------------------------------------------------------
# compat patches: this walrus build accepts only ONE sync-wait per instruction
# ----------------------------------------------------------------------------

def _patched_drain_and_barrier(self, tick_clock, wait_clock):
    nc = self.nc
    gc = tick_clock.global_clock
    n = len(gc)
    for p in range(n):
        t = gc[p]
        if t <= 0:
            continue
        vec = VectorClock([0] * n)
        vec.require_at_least(p, t)
        nop = nc.sync.nop(nofuse=True, hint=f"drain_wait_p{p}")
        wait_clock.add_sem_waits(nop.ins, ScopedClock({None: vec}))
    nc.sync.drain(fusable=False)
    nc.all_engine_barrier()
    assert self.sems is not None
    popped = nc._tile_sem_poison_stack.pop()
    assert popped is self._sem_poison
    nc.clear_and_free_semaphores(list(self.sems.allocated().values()))
    nc.all_engine_barrier()


def _split_multi_waits(bir_json: bytes) -> bytes:
    bj = json.loads(bir_json)
    n_split = 0
    for fn in bj.get("functions", []):
        for blk in fn.get("blocks", []):
            out = []
            for inst in blk.get("instructions", []):
                si = inst.get("sync_info")
                waits = si.get("on_wait", []) if si else []
                if len(waits) > 1:
                    for i, w in enumerate(waits[:-1]):
                        out.append({
                            "debug": inst.get("debug"),
                            "engine": inst["engine"],
                            "ins": [], "outs": [],
                            "name": f"{inst['name']}_sw{i}",
                            "opcode": "NoOp",
                            "sync_info": {"on_update": [], "on_wait": [w]},
                            "text_hint": "split_wait",
                        })
                        n_split += 1
                    si["on_wait"] = [waits[-1]]
                out.append(inst)
            blk["instructions"] = out
    if n_split:
        return json.dumps(bj).encode()
    return bir_json


_ORIG_COMPILE = bass_utils.compile_bir_kernel


def _patched_compile_bir_kernel(bir_json, tmpdir, neff_name="file.neff"):
    return _ORIG_COMPILE(_split_multi_waits(bir_json), tmpdir, neff_name)


def _apply_compat():
    tile.TileContext._drain_and_barrier = _patched_drain_and_barrier
    bass_utils.compile_bir_kernel = _patched_compile_bir_kernel
    bass2jax.compile_bir_kernel = _patched_compile_bir_kernel


_apply_compat()

# ----------------------------------------------------------------------------
# device program (SPMD: identical program, per-core data)
# ----------------------------------------------------------------------------

_NC_CACHE = None


def _build_program():
    nc = bass.Bass()
    xt8 = nc.declare_dram_parameter("xt8", [P, KTP, 2, S], F8, isOutput=False)
    wqk8 = nc.declare_dram_parameter("wqk8", [P, KTP, 2, 512], F8, isOutput=False)
    wvg8 = nc.declare_dram_parameter("wvg8", [P, KTP, 2, 272], F8, isOutput=False)
    wo8 = nc.declare_dram_parameter("wo8", [64, NH, D], F8, isOutput=False)
    rv = nc.declare_dram_parameter("rv", [P, ST], F32, isOutput=False)
    rv4 = nc.declare_dram_parameter("rv4", [P, 64], F32, isOutput=False)
    qn = nc.declare_dram_parameter("qn", [P, 1], F32, isOutput=False)
    kn = nc.declare_dram_parameter("kn", [P, 1], F32, isOutput=False)
    ind = nc.declare_dram_parameter("ind", [P, 2], BF16, isOutput=False)
    ind2 = nc.declare_dram_parameter("ind2", [2, P], BF16, isOutput=False)
    outp = nc.declare_dram_parameter("out_p", [S, D], BF16, isOutput=True)

    with TileContext(nc) as tc:
        with tc.tile_pool(name="big", bufs=1) as big, \
             tc.tile_pool(name="work", bufs=4) as work, \
             tc.tile_pool(name="wex", bufs=9) as wex, \
             tc.tile_pool(name="wsb", bufs=4) as wsb, \
             tc.tile_pool(name="wot", bufs=4) as wot, \
             tc.tile_pool(name="psc", bufs=2, space="PSUM") as psc, \
             tc.tile_pool(name="ppo", bufs=1, space="PSUM") as ppo:

            # ---- resident inputs (weights first; x chunked by s-range so
            # the first projection iterations start early; wo8 needed last)
            wqks = big.tile([P, KTP, 2, 512], F8)
            nc.sync.dma_start(out=wqks[:], in_=wqk8[:, :, :, :])
            xts = big.tile([P, KTP, 2, S], F8)
            for xc in range(4):
                nc.sync.dma_start(
                    out=xts[:, :, :, 512 * xc:512 * xc + 512],
                    in_=xt8[:, :, :, 512 * xc:512 * xc + 512])
            wvgs = big.tile([P, KTP, 2, 272], F8)
            nc.sync.dma_start(out=wvgs[:], in_=wvg8[:, :, :, :])
            wos = big.tile([64, NH, D], F8)
            nc.gpsimd.dma_start(out=wos[:], in_=wo8[:, :, :])
            # tiny scale/selector params ride the gpsimd DMA queue so they
            # land immediately instead of behind the big weight DMAs on SP
            qns = big.tile([P, 1], F32)
            nc.gpsimd.dma_start(out=qns[:], in_=qn[:, :])
            kns = big.tile([P, 1], F32)
            nc.gpsimd.dma_start(out=kns[:], in_=kn[:, :])
            inds = big.tile([P, 2], BF16)
            nc.gpsimd.dma_start(out=inds[:], in_=ind[:, :])
            ind2s = big.tile([2, P], BF16)
            nc.gpsimd.dma_start(out=ind2s[:], in_=ind2[:, :])
            rvs = big.tile([P, ST], F32)
            nc.gpsimd.dma_start(out=rvs[:], in_=rv[:, :])
            rv4s = big.tile([P, 64], F32)
            nc.gpsimd.dma_start(out=rv4s[:], in_=rv4[:, :])

            ident = big.tile([P, P], F32)
            make_identity(nc, ident[:])
            epsb = big.tile([P, 1], F32)
            nc.vector.memset(epsb[:], EPS)
            eps64 = big.tile([P, 1], F32)
            nc.vector.memset(eps64[:], HD * EPS)
            expbb = big.tile([P, 1], F32)
            nc.vector.memset(expbb[:], -EXPB)

            # ---- resident intermediates
            qkT = big.tile([P, 4, S], BF16)          # mt0,1=q(pair0,1) mt2,3=k
            vbuf = big.tile([P, ST // 2, 2, NH, 68], F8)  # [sk%128,sktp,e,j,hd+1]
            nc.vector.memset(vbuf[:, :, :, :, 64:65], 1.0)
            obuf = big.tile([64, NH, S], F8)         # gated attn outT per head
            gnat = big.tile([P, 16, 4], F32)         # tanh(r*gate/2) [t, j]
            gnat3 = big.tile([P, 16, 4], F32)        # 16*(1+tanh) = 32*sigmoid
            gstage = big.tile([P, 16, 4], F32)       # raw gate rows
            rkb = big.tile([P, 64], F32)             # 1/rms(k), col=4skt+j
            rkb64 = big.tile([P, 64], F32)           # rkb/64 (dve exp scale)
            ssum = big.tile([2, 2, S], F32)          # q sumsq rows per mt
            rqb = big.tile([2, 2, S], BF16)          # q scale rows per mt
            tmpq = big.tile([2, 2, S], F32)
            tmpk = big.tile([P, 64], F32)
            graw = big.tile([P, 64], F32)
            rs = big.tile([P, 16], F32)              # recip softmax sums
            crb = big.tile([P, 16], F32)             # 32*gate*rs per sq-tile

            # po layout: [bank(4), slot(4), col(128)]; slice q -> [q//4, q%4]
            # pre-attention scratch carved from the same banks:
            #   gate psum = pot[:, 0, 0, 0:64]; k-sumsq psum = pot[:, 0, 1, 0:64]
            pot = ppo.tile([P, 4, 512], F32, tag="po")

            # k-norm column layout: kcol(skt, j) = 32*(j//2) + 2*skt + (j%2)
            # so each k head-pair's stats occupy one contiguous 32-col half.
            def emit_sums(mt, ch, sqt):
                if mt < 2:  # q: row-layout sums [2, 512]
                    prf = proj_psum()
                    nc.tensor.matmul(prf(512)[0:2], inds[:], sqt[:],
                                     start=True, stop=True)
                    nc.vector.tensor_copy(
                        out=ssum[:, mt, 512 * ch:512 * ch + 512],
                        in_=prf(512)[0:2])
                else:  # k: column sums
                    for sl in range(4):
                        skt = ch * 4 + sl
                        c0 = 128 + 32 * (mt - 2) + 2 * skt
                        nc.tensor.matmul(
                            pot[:, 0, c0:c0 + 2],
                            sqt[:, 128 * sl:128 * sl + 128], inds[:],
                            start=True, stop=True)

            # projection iteration; elementwise on ACT only where allowed
            # (anything on the ACT FIFO ahead of the exps delays attention).
            # Projection psum rotates over 4 slots: the 2-deep "sc" ring plus
            # pot banks 2/3, which attention doesn't touch until its q>=8
            # attnV writes (far later, ordered by the tile framework).
            pend = []
            pslot = [0]

            def proj_psum():
                s = pslot[0] = 2 + (pslot[0] + 1) % 2
                return lambda n, s=s: pot[:, s, 0:n]

            def emit_proj(mt, ch, act_ok):
                if act_ok:
                    pqt = psc.tile([P, 1024], F32, tag="sc", name="pqt")
                    pqf = lambda n: pqt[:, 0:n]
                else:
                    pqf = proj_psum()
                for ktp in range(KTP):
                    nc.tensor.matmul(
                        pqf(512),
                        wqks[:, ktp, :, 128 * mt:128 * mt + 128],
                        xts[:, ktp, :, 512 * ch:512 * ch + 512],
                        start=(ktp == 0), stop=(ktp == KTP - 1),
                        perf_mode=DR)
                sc = qns if mt < 2 else kns
                qsl = qkT[:, mt, 512 * ch:512 * ch + 512]
                if act_ok:
                    nc.scalar.activation(qsl, pqf(512), AF.Copy,
                                         scale=sc[:])
                else:
                    nc.vector.tensor_scalar(qsl, pqf(512), sc[:],
                                            None, op0=ALU.mult)
                # squares from the bf16 copy (sbuf 2-byte: fast DVE path)
                sqt = work.tile([P, 512], BF16, tag="sq")
                nc.vector.tensor_tensor(sqt[:], qsl, qsl, ALU.mult)
                pend.append((mt, ch, sqt))
                if len(pend) > 2:
                    emit_sums(*pend.pop(0))

            def emit_khalf(half):
                sl = slice(32 * half, 32 * half + 32)
                psl = slice(128 + 32 * half, 160 + 32 * half)
                nc.scalar.activation(tmpk[:, sl], pot[:, 0, psl],
                                     AF.Ln, bias=epsb[:], scale=1.0 / HD)
                nc.scalar.activation(rkb[:, sl], tmpk[:, sl], AF.Exp,
                                     scale=-0.5)
                nc.vector.tensor_scalar(rkb64[:, sl], rkb[:, sl],
                                        1.0 / 64.0, None, op0=ALU.mult)

            def emit_qscale(mt, late=False):
                for chq in range(4):
                    if late:
                        pbct = psc.tile([P, 1024], F32, tag="sc", name="pbct")
                        pbc = lambda n: pbct[:, 0:n]
                    else:
                        pbc = proj_psum()
                    nc.tensor.matmul(
                        pbc(512), ind2s[:],
                        rqb[:, mt, 512 * chq:512 * chq + 512],
                        start=True, stop=True)
                    nc.vector.tensor_tensor(
                        qkT[:, mt, 512 * chq:512 * chq + 512],
                        qkT[:, mt, 512 * chq:512 * chq + 512],
                        pbc(512), ALU.mult)

            # ---- phase C part 1: mt0 (q heads 0/1) + mt2 (k heads 0/1),
            # interleaved by s-chunk to match the x DMA chunk arrival
            for mt, ch in [(m, c) for c in range(4) for m in (0, 2)]:
                emit_proj(mt, ch, act_ok=True)
            while pend:
                emit_sums(*pend.pop(0))
            # rq(mt0) in s-halves: the first scores need only half 0, so
            # the exp stream starts ~5us earlier
            for h2 in range(2):
                sl = slice(1024 * h2, 1024 * h2 + 1024)
                nc.scalar.activation(tmpq[:, 0, sl], ssum[:, 0, sl],
                                     AF.Ln, bias=eps64[0:2, :], scale=1.0)
                nc.scalar.activation(rqb[:, 0, sl], tmpq[:, 0, sl],
                                     AF.Exp, scale=-0.5)
                for chq in (2 * h2, 2 * h2 + 1):
                    pbc = proj_psum()
                    nc.tensor.matmul(
                        pbc(512), ind2s[:],
                        rqb[:, 0, 512 * chq:512 * chq + 512],
                        start=True, stop=True)
                    nc.vector.tensor_tensor(
                        qkT[:, 0, 512 * chq:512 * chq + 512],
                        qkT[:, 0, 512 * chq:512 * chq + 512],
                        pbc(512), ALU.mult)
            emit_khalf(0)

            # ---- phase E helper: v + gate projection (fp8 DoubleRow).
            # Emitted inside attention cycle 0, hidden under head-0's exps.
            def emit_E(t):
                pvf = proj_psum()
                for ktp in range(KTP):
                    nc.tensor.matmul(pvf(260),
                                     xts[:, ktp, :, 128 * t:128 * t + 128],
                                     wvgs[:, ktp, :, 0:260],
                                     start=(ktp == 0), stop=(ktp == KTP - 1),
                                     perf_mode=DR)
                vdst = vbuf[:, t // 2, t % 2, :, 0:64]
                nc.vector.tensor_scalar(vdst, pvf(260)[0:P, 0:256],
                                        rvs[:, t:t + 1], None, op0=ALU.mult)
                nc.vector.tensor_copy(out=gstage[:, t, :],
                                      in_=pvf(260)[0:P, 256:260])

            # ---- phase C part 2: mt3 (k heads 2/3) + mt1 (q heads 2/3).
            # All elementwise goes to the DVE: it drains during head-0/1
            # attention while the ACT is saturated with exp. The norm chains
            # for these heads are emitted inside the attention pipeline.
            for mt, ch in [(m, c) for m in (3, 1) for c in range(4)]:
                emit_proj(mt, ch, act_ok=False)
            while pend:
                emit_sums(*pend.pop(0))
            emit_khalf(1)
            nc.scalar.activation(tmpq[:, 1, :], ssum[:, 1, :], AF.Ln,
                                 bias=eps64[0:2, :], scale=1.0)
            nc.scalar.activation(rqb[:, 1, :], tmpq[:, 1, :], AF.Exp,
                                 scale=-0.5)
            emit_qscale(1)

            # ---- attention, software-pipelined over heads:
            # cycle jc: scores+exp for head jc interleaved with the finalize
            # of head jc-1 (gated copy, transpose into drained po slot,
            # 4-batched copyback to obuf); attnV for head jc runs as one
            # block at the end (ex tiles buffered in a deep ring), after all
            # of head jc-1's transposes, so the po banks swap owners cleanly.
            # DVE exp ladder: exp(t) ~= (1+t/64)^64 via 6 squarings (f32 for
            # the first three, bf16 after; the DVE runs 2-byte sbuf ops at
            # 2-4x). ~6x the ACT cost per tile, but it spends otherwise-idle
            # DVE cycles to shave the ACT-bound attention phase.
            def emit_exp_dve(ps, dst, c):
                e0 = work.tile([P, 1024], F32, tag="e0")
                nc.vector.tensor_scalar(e0[:], ps[:], rkb64[:, c:c + 1],
                                        C1EXP, op0=ALU.mult, op1=ALU.add)
                nc.vector.tensor_tensor(e0[:], e0[:], e0[:], ALU.mult)
                nc.vector.tensor_tensor(e0[:], e0[:], e0[:], ALU.mult)
                e1 = work.tile([P, 1024], BF16, tag="e1")
                with nc.allow_low_precision(reason="softmax wts are fp8"):
                    nc.vector.tensor_tensor(e1[:], e0[:], e0[:], ALU.mult)
                    nc.vector.tensor_tensor(e1[:], e1[:], e1[:], ALU.mult)
                    nc.vector.tensor_tensor(e1[:], e1[:], e1[:], ALU.mult)
                    nc.vector.tensor_tensor(dst, e1[:], e1[:], ALU.mult)

            exts = {}
            # attnV emission schedule: (sktp, q) lands at the first loop
            # index where its ex pair is computed AND po slot q's batch has
            # been copied back to obuf for the previous head (tp of slot q
            # precedes it in the PE FIFO, so no cross-engine deadlock).
            avsched = {}
            for sktp_ in range(8):
                for q_ in range(16):
                    avsched.setdefault(
                        max(2 * sktp_ + 1, 4 * (q_ // 4) + 3), []).append(
                            (sktp_, q_))
            for v_ in avsched.values():
                v_.sort()
            # cycle 0 variant: pot banks 2/3 double as phase-E psum slots, so
            # the q>=8 attnV (which overwrites them) waits for the last E tile
            avsched0 = {}
            for sktp_ in range(8):
                for q_ in range(16):
                    avsched0.setdefault(
                        15 if q_ >= 8 else max(2 * sktp_ + 1,
                                               4 * (q_ // 4) + 3), []).append(
                            (sktp_, q_))
            for v_ in avsched0.values():
                v_.sort()
            def emit_scores_exp(j, skt, h):
                mtq, mtk = j // 2, 2 + j // 2
                a = 64 * (j % 2)
                if (j, skt // 2) not in exts:
                    exts[(j, skt // 2)] = wex.tile(
                        [P, 2, S], F8, tag="ex", name="ext")
                ext = exts[(j, skt // 2)]
                ps = psc.tile([P, 1024], F32, tag="sc")
                for c2 in range(2):
                    q0 = 1024 * h + 512 * c2
                    nc.tensor.matmul(
                        ps[:, 512 * c2:512 * c2 + 512],
                        qkT[a:a + 64, mtk, 128 * skt:128 * skt + 128],
                        qkT[a:a + 64, mtq, q0:q0 + 512],
                        start=True, stop=True)
                col = 32 * (j // 2) + 2 * skt + (j % 2)
                nc.scalar.activation(
                    ext[:, skt % 2, 1024 * h:1024 * h + 1024], ps[:],
                    AF.Exp, bias=expbb[:], scale=rkb[:, col:col + 1])

            def emit_fin(jf, q, act_mix):
                posb = wsb.tile([P, 64], F32, tag="posb")
                qo = 128 * (q % 4)
                src = pot[:, q // 4, qo:qo + 64]
                if act_mix and q % 2 == 0:
                    nc.scalar.activation(posb[:], src, AF.Copy,
                                         scale=crb[:, q:q + 1])
                else:
                    nc.vector.tensor_scalar(posb[:], src, crb[:, q:q + 1],
                                            None, op0=ALU.mult)
                # transpose into the just-drained po slot
                nc.tensor.transpose(pot[0:64, q // 4, qo:qo + 128],
                                    posb[:], ident[:])
                if q % 4 == 3:
                    b = q // 4
                    dst = obuf[:, jf, 512 * b:512 * b + 512]
                    srcq = pot[0:64, b, 0:512]
                    if act_mix and b % 2 == 1:
                        nc.scalar.activation(dst, srcq, AF.Copy)
                    else:
                        nc.vector.tensor_copy(out=dst, in_=srcq)

            def emit_attnv(j, sktp, q):
                nc.tensor.matmul(
                    pot[:, q // 4, 128 * (q % 4):128 * (q % 4) + 65],
                    exts[(j, sktp)][:, :, 128 * q:128 * q + 128],
                    vbuf[:, sktp, :, j, 0:65],
                    start=(sktp == 0), stop=(sktp == 7), perf_mode=DR)

            def emit_oproj(t):
                ot = wot.tile([P, 1024], BF16, tag="ot")
                pp = psc.tile([P, 1024], F32, tag="sc")
                for nh in range(2):
                    for jp in range(2):
                        nc.tensor.matmul(
                            pp[:, 512 * nh:512 * nh + 512],
                            obuf[:, 2 * jp:2 * jp + 2,
                                 128 * t:128 * t + 128],
                            wos[:, 2 * jp:2 * jp + 2,
                                512 * nh:512 * nh + 512],
                            start=(jp == 0), stop=(jp == 1), perf_mode=DR)
                if t >= 4 and t % 2 == 0:
                    nc.scalar.activation(ot[:], pp[:], AF.Copy,
                                         scale=2.0 ** -10)
                else:
                    nc.vector.tensor_scalar(ot[:], pp[:], 2.0 ** -10,
                                            None, op0=ALU.mult)
                if t % 2 == 0:
                    nc.sync.dma_start(out=outp[128 * t:128 * t + 128, :],
                                      in_=ot[:])
                else:
                    nc.gpsimd.dma_start(out=outp[128 * t:128 * t + 128, :],
                                        in_=ot[:])

            # heads 0-2: scores+exp of head j over the finalize of head j-1
            for jc in range(3):
                j = jc
                jf = jc - 1 if jc > 0 else None
                if jc == 1:
                    # gate: 32*sigmoid(rg) = 32/(1+e^-rg); rv4 is negated on
                    # the host so Exp (already-loaded table set) suffices
                    nc.vector.tensor_tensor(graw[:], gstage[:], rv4s[:],
                                            ALU.mult)
                    nc.scalar.activation(gnat[:], graw[:], AF.Exp)
                    nc.vector.tensor_scalar(gnat[:], gnat[:], 1.0, None,
                                            op0=ALU.add)
                    nc.vector.reciprocal(gnat3[:], gnat[:])
                    nc.vector.tensor_scalar(gnat3[:], gnat3[:], 32.0, None,
                                            op0=ALU.mult)
                if jf is not None:
                    nc.vector.reciprocal(rs[:], pot[:, :, 64:512:128])
                    nc.vector.tensor_tensor(crb[:], rs[:], gnat3[:, :, jf],
                                            ALU.mult)
                for skt in range(ST):
                    emit_scores_exp(j, skt, 0)
                    emit_scores_exp(j, skt, 1)
                    if jf is not None:
                        emit_fin(jf, skt, False)
                    if jc == 0:
                        emit_E(skt)
                    sched = avsched0 if jc == 0 else avsched
                    for sktp, q in sched.get(skt, []):
                        emit_attnv(j, sktp, q)

            # head 3, pass A: h=0 exps cover attnV for sq-tiles 0-7;
            # head-2 finalize interleaves as usual
            nc.vector.reciprocal(rs[:], pot[:, :, 64:512:128])
            nc.vector.tensor_tensor(crb[:], rs[:], gnat3[:, :, 2],
                                    ALU.mult)
            for skt in range(ST):
                emit_scores_exp(3, skt, 0)
                emit_fin(2, skt, False)
                for sktp, q in avsched.get(skt, []):
                    if q < 8:
                        emit_attnv(3, sktp, q)
            # pass B: h=1 exps; head-3's low sq-tiles finalize and the first
            # half of o_proj runs underneath them
            nc.vector.reciprocal(rs[:, 0:8], pot[:, 0:2, 64:512:128])
            nc.vector.tensor_tensor(crb[:, 0:8], rs[:, 0:8],
                                    gnat3[:, 0:8, 3], ALU.mult)
            for idx in range(ST):
                emit_scores_exp(3, idx, 1)
                if idx % 2 == 0:
                    emit_fin(3, idx // 2, False)
                else:
                    for q in range(8, 16):
                        emit_attnv(3, (idx - 1) // 2, q)
                if idx >= 7 and idx % 2 == 1:
                    emit_oproj((idx - 7) // 2)
            # pass C: high sq-tiles of head 3 + the rest of o_proj
            nc.vector.reciprocal(rs[:, 8:16], pot[:, 2:4, 64:512:128])
            nc.vector.tensor_tensor(crb[:, 8:16], rs[:, 8:16],
                                    gnat3[:, 8:16, 3], ALU.mult)
            oq = [4], [5], [6], [7, 8], [9], [10], [11], [12, 13, 14, 15]
            for i, q in enumerate(range(8, 16)):
                emit_fin(3, q, True)
                for t in oq[i]:
                    emit_oproj(t)
    return nc


def _get_program():
    global _NC_CACHE
    if _NC_CACHE is None:
        _NC_CACHE = _build_program()
    return _NC_CACHE


# ----------------------------------------------------------------------------
# host wrapper
# ----------------------------------------------------------------------------

def _prep_inputs(x, prenorm_w, qkv_w, gate_w, o_w, q_norm_w, k_norm_w):
    x = np.asarray(x, np.float32)
    pw = np.asarray(prenorm_w, np.float32)
    qkv_w = np.asarray(qkv_w, np.float32)
    gate_w = np.asarray(gate_w, np.float32)
    o_w = np.asarray(o_w, np.float32)
    qw = qkv_w[0:D] * pw[None, :]
    kw = qkv_w[D:2 * D] * pw[None, :]
    vw = qkv_w[2 * D:3 * D] * pw[None, :]
    gw = gate_w * pw[None, :]

    r = 1.0 / np.sqrt(np.mean(x * x, axis=-1) + EPS)      # [B, S]
    ind = np.zeros((P, 2), BF)
    ind[0:64, 0] = 1
    ind[64:128, 1] = 1
    ind2 = np.zeros((2, P), BF)
    ind2[0, 0:64] = 1
    ind2[1, 64:128] = 1
    qn = (np.tile(np.asarray(q_norm_w, np.float32), 2) / 32.0)[:, None]
    kn = (np.tile(np.asarray(k_norm_w, np.float32), 2) / 32.0)[:, None]

    in_maps = []
    for c in range(8):
        b, hg = c // 4, c % 4
        hsl = slice(256 * hg, 256 * hg + 256)
        xtc = np.ascontiguousarray(
            x[b].T.reshape(KTP, 2, P, S).transpose(2, 0, 1, 3)).astype(F8NP)
        wqk = np.concatenate([qw[hsl], kw[hsl]], 0).T * 32.0  # [1024, 512]
        wqkc = np.ascontiguousarray(
            wqk.reshape(KTP, 2, P, 512).transpose(2, 0, 1, 3)).astype(F8NP)
        wvg = np.concatenate([vw[hsl], gw[4 * hg:4 * hg + 4]], 0).T * 32.0
        wvgp = np.zeros((D, 272), np.float32)
        wvgp[:, 0:260] = wvg
        wvgc = np.ascontiguousarray(
            wvgp.reshape(KTP, 2, P, 272).transpose(2, 0, 1, 3)).astype(F8NP)
        wo = o_w[:, hsl].T.reshape(NH, 64, D).transpose(1, 0, 2) * 32.0
        woc = np.ascontiguousarray(wo).astype(F8NP)
        rvc = np.ascontiguousarray(
            r[b].reshape(ST, P).T / 32.0).astype(np.float32)
        rv4c = np.ascontiguousarray(np.repeat(-rvc, 4, axis=1))
        in_maps.append({
            "xt8": xtc, "wqk8": wqkc, "wvg8": wvgc, "wo8": woc,
            "rv": rvc, "rv4": rv4c, "qn": qn.astype(np.float32),
            "kn": kn.astype(np.float32), "ind": ind, "ind2": ind2,
        })
    return in_maps


_RUNNER = None


def _get_runner():
    """Build the sharded PJRT executable ONCE and reuse it across calls
    (run_bass_kernel_spmd re-traces/re-compiles on every invocation)."""
    global _RUNNER
    if _RUNNER is not None:
        return _RUNNER
    import jax
    import concourse.mybir as _mybir
    from concourse.bass2jax import (_bass_exec_p, partition_id_tensor,
                                    install_neuronx_cc_hook, Mesh,
                                    PartitionSpec, shard_map)
    install_neuronx_cc_hook()
    nc = _get_program()
    n_cores = 8
    partition_name = (nc.partition_id_tensor.name
                      if nc.partition_id_tensor else None)
    in_names, out_names, out_avals, zero_outs = [], [], [], []
    for alloc in nc.m.functions[0].allocations:
        if not isinstance(alloc, _mybir.MemoryLocationSet):
            continue
        name = alloc.memorylocations[0].name
        if alloc.kind == "ExternalInput":
            if name != partition_name:
                in_names.append(name)
        elif alloc.kind == "ExternalOutput":
            shape = tuple(alloc.tensor_shape)
            dtype = _mybir.dt.np(alloc.dtype)
            out_names.append(name)
            out_avals.append(jax.core.ShapedArray(shape, dtype))
            zero_outs.append(np.zeros(shape, dtype))
    n_params = len(in_names)
    n_outs = len(out_avals)
    all_in = list(in_names) + list(out_names)
    if partition_name is not None:
        all_in.append(partition_name)
    donate = tuple(range(n_params, n_params + n_outs))

    def _body(*args):
        operands = list(args)
        if partition_name is not None:
            operands.append(partition_id_tensor())
        return tuple(_bass_exec_p.bind(
            *operands, out_avals=tuple(out_avals), in_names=tuple(all_in),
            out_names=tuple(out_names), lowering_input_output_aliases=(),
            sim_require_finite=True, sim_require_nnan=True, nc=nc))

    devices = jax.devices()[:n_cores]
    mesh = Mesh(np.asarray(devices), ("core",))
    sharded = jax.jit(
        shard_map(_body, mesh=mesh,
                  in_specs=(PartitionSpec("core"),) * (n_params + n_outs),
                  out_specs=(PartitionSpec("core"),) * n_outs,
                  check_rep=False),
        donate_argnums=donate, keep_unused=True)
    _RUNNER = (sharded, in_names, out_names, out_avals, zero_outs, n_cores)
    return _RUNNER


def kernel(x, prenorm_w, qkv_w, gate_w, o_w, q_norm_w, k_norm_w):
    sharded, in_names, out_names, out_avals, zero_outs, n_cores = _get_runner()
    in_maps = _prep_inputs(x, prenorm_w, qkv_w, gate_w, o_w,
                           q_norm_w, k_norm_w)
    concat_in = [np.concatenate([in_maps[c][nm] for c in range(n_cores)], 0)
                 for nm in in_names]
    concat_zeros = [np.zeros((n_cores * z.shape[0], *z.shape[1:]), z.dtype)
                    for z in zero_outs]
    out_arrs = sharded(*concat_in, *concat_zeros)
    oi = out_names.index("out_p")
    op = np.asarray(out_arrs[oi]).astype(np.float32).reshape(
        n_cores, *out_avals[oi].shape)
    outs = [op[c] for c in range(n_cores)]
    x = np.asarray(x, np.float32)
    y0 = x[0] + outs[0] + outs[1] + outs[2] + outs[3]
    y1 = x[1] + outs[4] + outs[5] + outs[6] + outs[7]
    return np.stack([y0, y1]).astype(np.float32)


# revision 61
# speedup vs baseline: 1.7154x; 1.0117x over previous
"""GatedAttention Trainium2 kernel (8 NeuronCores, tensor-parallel over (batch, head-group)).

Sharding: core c handles batch b=c//4 and heads 4*(c%4)..4*(c%4)+3.
Each core computes qkv/gate projections for its heads from x[b], per-head
QK-RMS-norm + softmax attention + sigmoid gating, and a row-split o_proj
partial [S, D] (bf16). Host sums the 4 partials per batch + residual.

Key structure:
- All big matmuls run in fp8e4m3 with DoubleRow (contraction pairs packed
  into the free dim of both operands: [Ki, 2, M] x [Ki, 2, N]). Weights are
  pre-scaled x32 on host so fp8 sees ~unit-variance values; descales fold
  into copyback scales.
- attnV emits [sq, hd] tiles (out free = 65) instead of [hd, sq]: softmax
  sums + sigmoid gate become per-partition scalars. A per-sq-tile PE
  transpose rebuilds the [hd, sq] layout o_proj needs, writing into the
  just-drained po PSUM slot.
- exp is the elementwise wall (16.8M elems/core): batched to N=1024 per
  instruction and split between ACT (native Exp) and a custom single-pass
  8-stage DVE op computing exp(t) ~= (1 + t/64)^64.
- softmax runs without max-subtraction; a uniform -3*ln2 bias keeps exp
  outputs under the fp8e4m3 max (e^(8-2.079)=372<448; |s|<=8 by
  Cauchy-Schwarz after QK RMS norm). The bias cancels in normalization.
- v carries an all-ones column so attnV also yields the softmax sums.
"""

import json
import math

import numpy as np
import ml_dtypes

import concourse.bass as bass
import concourse.bass_utils as bass_utils
import concourse.bass2jax as bass2jax
import concourse.mybir as mybir
import concourse.tile as tile
import concourse.dve_ops as dve_ops
from concourse.dve_ops import DveOp
from concourse.dve_spec import Spec, Src0, C0, C1, sq as dve_sq
from concourse.tile import TileContext
from concourse.masks import make_identity
from concourse.vector_clock import ScopedClock, VectorClock

F32 = mybir.dt.float32
BF16 = mybir.dt.bfloat16
F8 = mybir.dt.float8e4
AF = mybir.ActivationFunctionType
ALU = mybir.AluOpType
DR = mybir.MatmulPerfMode.DoubleRow
BF = ml_dtypes.bfloat16
F8NP = mybir.dt.np(mybir.dt.float8e4)

B, S, D = 2, 2048, 1024
NH_TOT, HD = 16, 64
NH = 4            # heads per core
EPS = 1e-5
P = 128
ST = S // P       # 16 s-tiles
KTP = 4           # d-dim pair-tiles (4 x (2x128))
EXPB = 3 * math.log(2.0)          # uniform score bias (cancels in softmax)
C1EXP = 1.0 - EXPB / 64.0         # dve exp: a = s*rk/64 + C1EXP

# engine split knobs (tuned against sim engine-busy readout)
# NOTE: this container's walrus cannot codegen custom-DVE ops ("ISA wrong
# length" even for the production RECIPROCAL_APPROX_FAST), so exp runs
# entirely on ACT and everything else moves to DVE.
EXP_ACT_OF_32 = 32   # of the 32 exp half-tiles per head, this many on ACT

# ----------------------------------------------------------------------------
# custom DVE op: exp(t) ~= (1 + t/64)^64, one pass, 8 uop stages
# ----------------------------------------------------------------------------


def _ref_exp64(in0, in1, s0, s1, imm2):
    a = in0.astype(np.float32) * np.asarray(s0, np.float32) + np.float32(s1)
    for _ in range(6):
        a = a * a
    return a


EXP64_ANT = DveOp(
    "EXP64_ANT",
    Spec(
        body=dve_sq(dve_sq(dve_sq(dve_sq(dve_sq(dve_sq(Src0 * C0 + C1)))))),
        reference=_ref_exp64,
    ),
    subdim=False,
    uops_sha={"v3": "8299cc4e9a89acf1", "v4": "df7b3d1456faeb1a"},
)


def _register_exp_op():
    if EXP64_ANT.name in dve_ops.CUSTOM_DVE_SPECS:
        return
    row = max(dve_ops._SUB_OPCODE_FOR_NAME.values()) + 1
    assert row < 0x20
    dve_ops.OPS.append(EXP64_ANT)
    dve_ops.CUSTOM_DVE_SPECS[EXP64_ANT.name] = EXP64_ANT.spec
    dve_ops._SUB_OPCODE_FOR_NAME[EXP64_ANT.name] = row


_register_exp_op()

# ----------------------------------------------------------------------------
# compat patches: this walrus build accepts only ONE sync-wait per instruction
# ----------------------------------------------------------------------------

def _patched_drain_and_barrier(self, tick_clock, wait_clock):
    nc = self.nc
    gc = tick_clock.global_clock
    n = len(gc)
    for p in range(n):
        t = gc[p]
        if t <= 0:
            continue
        vec = VectorClock([0] * n)
        vec.require_at_least(p, t)
        nop = nc.sync.nop(nofuse=True, hint=f"drain_wait_p{p}")
        wait_clock.add_sem_waits(nop.ins, ScopedClock({None: vec}))
    nc.sync.drain(fusable=False)
    nc.all_engine_barrier()
    assert self.sems is not None
    popped = nc._tile_sem_poison_stack.pop()
    assert popped is self._sem_poison
    nc.clear_and_free_semaphores(list(self.sems.allocated().values()))
    nc.all_engine_barrier()


def _split_multi_waits(bir_json: bytes) -> bytes:
    bj = json.loads(bir_json)
    n_split = 0
    for fn in bj.get("functions", []):
        for blk in fn.get("blocks", []):
            out = []
            for inst in blk.get("instructions", []):
                si = inst.get("sync_info")
                waits = si.get("on_wait", []) if si else []
                if len(waits) > 1:
                    for i, w in enumerate(waits[:-1]):
                        out.append({
                            "debug": inst.get("debug"),
                            "engine": inst["engine"],
                            "ins": [], "outs": [],
                            "name": f"{inst['name']}_sw{i}",
                            "opcode": "NoOp",
                            "sync_info": {"on_update": [], "on_wait": [w]},
                            "text_hint": "split_wait",
                        })
                        n_split += 1
                    si["on_wait"] = [waits[-1]]
                out.append(inst)
            blk["instructions"] = out
    if n_split:
        return json.dumps(bj).encode()
    return bir_json


_ORIG_COMPILE = bass_utils.compile_bir_kernel


def _patched_compile_bir_kernel(bir_json, tmpdir, neff_name="file.neff"):
    return _ORIG_COMPILE(_split_multi_waits(bir_json), tmpdir, neff_name)


def _apply_compat():
    tile.TileContext._drain_and_barrier = _patched_drain_and_barrier
    bass_utils.compile_bir_kernel = _patched_compile_bir_kernel
    bass2jax.compile_bir_kernel = _patched_compile_bir_kernel


_apply_compat()

# ----------------------------------------------------------------------------
# device program (SPMD: identical program, per-core data)
# ----------------------------------------------------------------------------

_NC_CACHE = None


def _build_program():
    nc = bass.Bass()
    xt8 = nc.declare_dram_parameter("xt8", [P, KTP, 2, S], F8, isOutput=False)
    wqk8 = nc.declare_dram_parameter("wqk8", [P, KTP, 2, 512], F8, isOutput=False)
    wvg8 = nc.declare_dram_parameter("wvg8", [P, KTP, 2, 272], F8, isOutput=False)
    wo8 = nc.declare_dram_parameter("wo8", [64, NH, D], F8, isOutput=False)
    rv = nc.declare_dram_parameter("rv", [P, ST], F32, isOutput=False)
    rv4 = nc.declare_dram_parameter("rv4", [P, 64], F32, isOutput=False)
    qn = nc.declare_dram_parameter("qn", [P, 1], F32, isOutput=False)
    kn = nc.declare_dram_parameter("kn", [P, 1], F32, isOutput=False)
    ind = nc.declare_dram_parameter("ind", [P, 2], BF16, isOutput=False)
    ind2 = nc.declare_dram_parameter("ind2", [2, P], BF16, isOutput=False)
    outp = nc.declare_dram_parameter("out_p", [S, D], BF16, isOutput=True)

    with TileContext(nc) as tc:
        with tc.tile_pool(name="big", bufs=1) as big, \
             tc.tile_pool(name="work", bufs=4) as work, \
             tc.tile_pool(name="wex", bufs=9) as wex, \
             tc.tile_pool(name="wsb", bufs=4) as wsb, \
             tc.tile_pool(name="wot", bufs=4) as wot, \
             tc.tile_pool(name="psc", bufs=2, space="PSUM") as psc, \
             tc.tile_pool(name="ppo", bufs=1, space="PSUM") as ppo:

            # ---- resident inputs (weights first; x chunked by s-range so
            # the first projection iterations start early; wo8 needed last)
            wqks = big.tile([P, KTP, 2, 512], F8)
            nc.scalar.dma_start(out=wqks[:], in_=wqk8[:, :, :, :])
            xts = big.tile([P, KTP, 2, S], F8)
            for xc in range(4):
                nc.sync.dma_start(
                    out=xts[:, :, :, 512 * xc:512 * xc + 512],
                    in_=xt8[:, :, :, 512 * xc:512 * xc + 512])
            wvgs = big.tile([P, KTP, 2, 272], F8)
            nc.sync.dma_start(out=wvgs[:], in_=wvg8[:, :, :, :])
            wos = big.tile([64, NH, D], F8)
            nc.gpsimd.dma_start(out=wos[:], in_=wo8[:, :, :])
            # tiny scale/selector params ride the gpsimd DMA queue so they
            # land immediately instead of behind the big weight DMAs on SP
            qns = big.tile([P, 1], F32)
            nc.gpsimd.dma_start(out=qns[:], in_=qn[:, :])
            kns = big.tile([P, 1], F32)
            nc.gpsimd.dma_start(out=kns[:], in_=kn[:, :])
            inds = big.tile([P, 2], BF16)
            nc.gpsimd.dma_start(out=inds[:], in_=ind[:, :])
            ind2s = big.tile([2, P], BF16)
            nc.gpsimd.dma_start(out=ind2s[:], in_=ind2[:, :])
            rvs = big.tile([P, ST], F32)
            nc.gpsimd.dma_start(out=rvs[:], in_=rv[:, :])
            rv4s = big.tile([P, 64], F32)
            nc.gpsimd.dma_start(out=rv4s[:], in_=rv4[:, :])

            ident = big.tile([P, P], F32)
            make_identity(nc, ident[:])
            epsb = big.tile([P, 1], F32)
            nc.vector.memset(epsb[:], EPS)
            eps64 = big.tile([P, 1], F32)
            nc.vector.memset(eps64[:], HD * EPS)
            expbb = big.tile([P, 1], F32)
            nc.vector.memset(expbb[:], -EXPB)

            # ---- resident intermediates
            qkT = big.tile([P, 4, S], BF16)          # mt0,1=q(pair0,1) mt2,3=k
            vbuf = big.tile([P, ST // 2, 2, NH, 68], F8)  # [sk%128,sktp,e,j,hd+1]
            nc.vector.memset(vbuf[:, :, :, :, 64:65], 1.0)
            obuf = big.tile([64, NH, S], F8)         # gated attn outT per head
            gnat = big.tile([P, 16, 4], F32)         # tanh(r*gate/2) [t, j]
            gnat3 = big.tile([P, 16, 4], F32)        # 16*(1+tanh) = 32*sigmoid
            gstage = big.tile([P, 16, 4], F32)       # raw gate rows
            rkb = big.tile([P, 64], F32)             # 1/rms(k), col=4skt+j
            rkb64 = big.tile([P, 64], F32)           # rkb/64 (dve exp scale)
            ssum = big.tile([2, 2, S], F32)          # q sumsq rows per mt
            rqb = big.tile([2, 2, S], BF16)          # q scale rows per mt
            tmpq = big.tile([2, 2, S], F32)
            tmpk = big.tile([P, 64], F32)
            graw = big.tile([P, 64], F32)
            rs = big.tile([P, 16], F32)              # recip softmax sums
            crb = big.tile([P, 16], F32)             # 32*gate*rs per sq-tile

            # po layout: [bank(4), slot(4), col(128)]; slice q -> [q//4, q%4]
            # pre-attention scratch carved from the same banks:
            #   gate psum = pot[:, 0, 0, 0:64]; k-sumsq psum = pot[:, 0, 1, 0:64]
            pot = ppo.tile([P, 4, 512], F32, tag="po")

            # k-norm column layout: kcol(skt, j) = 32*(j//2) + 2*skt + (j%2)
            # so each k head-pair's stats occupy one contiguous 32-col half.
            def emit_sums(mt, ch, sqt):
                if mt < 2:  # q: row-layout sums [2, 512]
                    prf = proj_psum()
                    nc.tensor.matmul(prf(512)[0:2], inds[:], sqt[:],
                                     start=True, stop=True)
                    nc.vector.tensor_copy(
                        out=ssum[:, mt, 512 * ch:512 * ch + 512],
                        in_=prf(512)[0:2])
                else:  # k: column sums
                    for sl in range(4):
                        skt = ch * 4 + sl
                        c0 = 128 + 32 * (mt - 2) + 2 * skt
                        nc.tensor.matmul(
                            pot[:, 0, c0:c0 + 2],
                            sqt[:, 128 * sl:128 * sl + 128], inds[:],
                            start=True, stop=True)

            # projection iteration; elementwise on ACT only where allowed
            # (anything on the ACT FIFO ahead of the exps delays attention).
            # Projection psum rotates over 4 slots: the 2-deep "sc" ring plus
            # pot banks 2/3, which attention doesn't touch until its q>=8
            # attnV writes (far later, ordered by the tile framework).
            pend = []
            pslot = [0]

            def proj_psum():
                s = pslot[0] = 2 + (pslot[0] + 1) % 2
                return lambda n, s=s: pot[:, s, 0:n]

            def emit_proj(mt, ch, act_ok):
                if act_ok:
                    pqt = psc.tile([P, 1024], F32, tag="sc", name="pqt")
                    pqf = lambda n: pqt[:, 0:n]
                else:
                    pqf = proj_psum()
                for ktp in range(KTP):
                    nc.tensor.matmul(
                        pqf(512),
                        wqks[:, ktp, :, 128 * mt:128 * mt + 128],
                        xts[:, ktp, :, 512 * ch:512 * ch + 512],
                        start=(ktp == 0), stop=(ktp == KTP - 1),
                        perf_mode=DR)
                sc = qns if mt < 2 else kns
                qsl = qkT[:, mt, 512 * ch:512 * ch + 512]
                if act_ok:
                    nc.scalar.activation(qsl, pqf(512), AF.Copy,
                                         scale=sc[:])
                else:
                    nc.vector.tensor_scalar(qsl, pqf(512), sc[:],
                                            None, op0=ALU.mult)
                # squares from the bf16 copy (sbuf 2-byte: fast DVE path)
                sqt = work.tile([P, 512], BF16, tag="sq")
                nc.vector.tensor_tensor(sqt[:], qsl, qsl, ALU.mult)
                pend.append((mt, ch, sqt))
                if len(pend) > 2:
                    emit_sums(*pend.pop(0))

            def emit_khalf(half):
                sl = slice(32 * half, 32 * half + 32)
                psl = slice(128 + 32 * half, 160 + 32 * half)
                nc.scalar.activation(tmpk[:, sl], pot[:, 0, psl],
                                     AF.Ln, bias=epsb[:], scale=1.0 / HD)
                nc.scalar.activation(rkb[:, sl], tmpk[:, sl], AF.Exp,
                                     scale=-0.5)
                nc.vector.tensor_scalar(rkb64[:, sl], rkb[:, sl],
                                        1.0 / 64.0, None, op0=ALU.mult)

            def emit_qscale(mt, late=False):
                for chq in range(4):
                    if late:
                        pbct = psc.tile([P, 1024], F32, tag="sc", name="pbct")
                        pbc = lambda n: pbct[:, 0:n]
                    else:
                        pbc = proj_psum()
                    nc.tensor.matmul(
                        pbc(512), ind2s[:],
                        rqb[:, mt, 512 * chq:512 * chq + 512],
                        start=True, stop=True)
                    nc.vector.tensor_tensor(
                        qkT[:, mt, 512 * chq:512 * chq + 512],
                        qkT[:, mt, 512 * chq:512 * chq + 512],
                        pbc(512), ALU.mult)

            # ---- phase C part 1: mt0 (q heads 0/1) + mt2 (k heads 0/1),
            # interleaved by s-chunk to match the x DMA chunk arrival
            for mt, ch in [(m, c) for c in range(4) for m in (0, 2)]:
                emit_proj(mt, ch, act_ok=True)
            while pend:
                emit_sums(*pend.pop(0))
            # rq(mt0) in s-halves: the first scores need only half 0, so
            # the exp stream starts ~5us earlier
            for h2 in range(2):
                sl = slice(1024 * h2, 1024 * h2 + 1024)
                nc.scalar.activation(tmpq[:, 0, sl], ssum[:, 0, sl],
                                     AF.Ln, bias=eps64[0:2, :], scale=1.0)
                nc.scalar.activation(rqb[:, 0, sl], tmpq[:, 0, sl],
                                     AF.Exp, scale=-0.5)
                for chq in (2 * h2, 2 * h2 + 1):
                    pbc = proj_psum()
                    nc.tensor.matmul(
                        pbc(512), ind2s[:],
                        rqb[:, 0, 512 * chq:512 * chq + 512],
                        start=True, stop=True)
                    nc.vector.tensor_tensor(
                        qkT[:, 0, 512 * chq:512 * chq + 512],
                        qkT[:, 0, 512 * chq:512 * chq + 512],
                        pbc(512), ALU.mult)
            emit_khalf(0)

            # ---- phase E helper: v + gate projection (fp8 DoubleRow).
            # Emitted inside attention cycle 0, hidden under head-0's exps.
            def emit_E(t):
                pvf = proj_psum()
                for ktp in range(KTP):
                    nc.tensor.matmul(pvf(260),
                                     xts[:, ktp, :, 128 * t:128 * t + 128],
                                     wvgs[:, ktp, :, 0:260],
                                     start=(ktp == 0), stop=(ktp == KTP - 1),
                                     perf_mode=DR)
                vdst = vbuf[:, t // 2, t % 2, :, 0:64]
                nc.vector.tensor_scalar(vdst, pvf(260)[0:P, 0:256],
                                        rvs[:, t:t + 1], None, op0=ALU.mult)
                nc.vector.tensor_copy(out=gstage[:, t, :],
                                      in_=pvf(260)[0:P, 256:260])

            # ---- phase C part 2: mt3 (k heads 2/3) + mt1 (q heads 2/3).
            # All elementwise goes to the DVE: it drains during head-0/1
            # attention while the ACT is saturated with exp. The norm chains
            # for these heads are emitted inside the attention pipeline.
            for mt, ch in [(m, c) for m in (3, 1) for c in range(4)]:
                emit_proj(mt, ch, act_ok=False)
            while pend:
                emit_sums(*pend.pop(0))
            emit_khalf(1)
            nc.scalar.activation(tmpq[:, 1, :], ssum[:, 1, :], AF.Ln,
                                 bias=eps64[0:2, :], scale=1.0)
            nc.scalar.activation(rqb[:, 1, :], tmpq[:, 1, :], AF.Exp,
                                 scale=-0.5)
            emit_qscale(1)

            # ---- attention, software-pipelined over heads:
            # cycle jc: scores+exp for head jc interleaved with the finalize
            # of head jc-1 (gated copy, transpose into drained po slot,
            # 4-batched copyback to obuf); attnV for head jc runs as one
            # block at the end (ex tiles buffered in a deep ring), after all
            # of head jc-1's transposes, so the po banks swap owners cleanly.
            # DVE exp ladder: exp(t) ~= (1+t/64)^64 via 6 squarings (f32 for
            # the first three, bf16 after; the DVE runs 2-byte sbuf ops at
            # 2-4x). ~6x the ACT cost per tile, but it spends otherwise-idle
            # DVE cycles to shave the ACT-bound attention phase.
            def emit_exp_dve(ps, dst, c):
                e0 = work.tile([P, 1024], F32, tag="e0")
                nc.vector.tensor_scalar(e0[:], ps[:], rkb64[:, c:c + 1],
                                        C1EXP, op0=ALU.mult, op1=ALU.add)
                nc.vector.tensor_tensor(e0[:], e0[:], e0[:], ALU.mult)
                nc.vector.tensor_tensor(e0[:], e0[:], e0[:], ALU.mult)
                e1 = work.tile([P, 1024], BF16, tag="e1")
                with nc.allow_low_precision(reason="softmax wts are fp8"):
                    nc.vector.tensor_tensor(e1[:], e0[:], e0[:], ALU.mult)
                    nc.vector.tensor_tensor(e1[:], e1[:], e1[:], ALU.mult)
                    nc.vector.tensor_tensor(e1[:], e1[:], e1[:], ALU.mult)
                    nc.vector.tensor_tensor(dst, e1[:], e1[:], ALU.mult)

            exts = {}
            # attnV emission schedule: (sktp, q) lands at the first loop
            # index where its ex pair is computed AND po slot q's batch has
            # been copied back to obuf for the previous head (tp of slot q
            # precedes it in the PE FIFO, so no cross-engine deadlock).
            avsched = {}
            for sktp_ in range(8):
                for q_ in range(16):
                    avsched.setdefault(
                        max(2 * sktp_ + 1, 4 * (q_ // 4) + 3), []).append(
                            (sktp_, q_))
            for v_ in avsched.values():
                v_.sort()
            # cycle 0 variant: pot banks 2/3 double as phase-E psum slots, so
            # the q>=8 attnV (which overwrites them) waits for the last E tile
            avsched0 = {}
            for sktp_ in range(8):
                for q_ in range(16):
                    avsched0.setdefault(
                        15 if q_ >= 8 else max(2 * sktp_ + 1,
                                               4 * (q_ // 4) + 3), []).append(
                            (sktp_, q_))
            for v_ in avsched0.values():
                v_.sort()
            def emit_scores_exp(j, skt, h):
                mtq, mtk = j // 2, 2 + j // 2
                a = 64 * (j % 2)
                if (j, skt // 2) not in exts:
                    exts[(j, skt // 2)] = wex.tile(
                        [P, 2, S], F8, tag="ex", name="ext")
                ext = exts[(j, skt // 2)]
                ps = psc.tile([P, 1024], F32, tag="sc")
                for c2 in range(2):
                    q0 = 1024 * h + 512 * c2
                    nc.tensor.matmul(
                        ps[:, 512 * c2:512 * c2 + 512],
                        qkT[a:a + 64, mtk, 128 * skt:128 * skt + 128],
                        qkT[a:a + 64, mtq, q0:q0 + 512],
                        start=True, stop=True)
                col = 32 * (j // 2) + 2 * skt + (j % 2)
                nc.scalar.activation(
                    ext[:, skt % 2, 1024 * h:1024 * h + 1024], ps[:],
                    AF.Exp, bias=expbb[:], scale=rkb[:, col:col + 1])

            def emit_fin(jf, q, act_mix):
                posb = wsb.tile([P, 64], F32, tag="posb")
                qo = 128 * (q % 4)
                src = pot[:, q // 4, qo:qo + 64]
                if act_mix and q % 2 == 0:
                    nc.scalar.activation(posb[:], src, AF.Copy,
                                         scale=crb[:, q:q + 1])
                else:
                    nc.vector.tensor_scalar(posb[:], src, crb[:, q:q + 1],
                                            None, op0=ALU.mult)
                # transpose into the just-drained po slot
                nc.tensor.transpose(pot[0:64, q // 4, qo:qo + 128],
                                    posb[:], ident[:])
                if q % 4 == 3:
                    b = q // 4
                    dst = obuf[:, jf, 512 * b:512 * b + 512]
                    srcq = pot[0:64, b, 0:512]
                    if act_mix and b % 2 == 1:
                        nc.scalar.activation(dst, srcq, AF.Copy)
                    else:
                        nc.vector.tensor_copy(out=dst, in_=srcq)

            def emit_attnv(j, sktp, q):
                nc.tensor.matmul(
                    pot[:, q // 4, 128 * (q % 4):128 * (q % 4) + 65],
                    exts[(j, sktp)][:, :, 128 * q:128 * q + 128],
                    vbuf[:, sktp, :, j, 0:65],
                    start=(sktp == 0), stop=(sktp == 7), perf_mode=DR)

            def emit_oproj(t):
                ot = wot.tile([P, 1024], BF16, tag="ot")
                pp = psc.tile([P, 1024], F32, tag="sc")
                for nh in range(2):
                    for jp in range(2):
                        nc.tensor.matmul(
                            pp[:, 512 * nh:512 * nh + 512],
                            obuf[:, 2 * jp:2 * jp + 2,
                                 128 * t:128 * t + 128],
                            wos[:, 2 * jp:2 * jp + 2,
                                512 * nh:512 * nh + 512],
                            start=(jp == 0), stop=(jp == 1), perf_mode=DR)
                if t >= 4 and t % 2 == 0:
                    nc.scalar.activation(ot[:], pp[:], AF.Copy,
                                         scale=2.0 ** -10)
                else:
                    nc.vector.tensor_scalar(ot[:], pp[:], 2.0 ** -10,
                                            None, op0=ALU.mult)
                if t % 2 == 0:
                    nc.sync.dma_start(out=outp[128 * t:128 * t + 128, :],
                                      in_=ot[:])
                else:
                    nc.gpsimd.dma_start(out=outp[128 * t:128 * t + 128, :],
                                        in_=ot[:])

            # heads 0-2: scores+exp of head j over the finalize of head j-1
            for jc in range(3):
                j = jc
                jf = jc - 1 if jc > 0 else None
                if jc == 1:
                    # gate: 32*sigmoid(rg) = 32/(1+e^-rg); rv4 is negated on
                    # the host so Exp (already-loaded table set) suffices
                    nc.vector.tensor_tensor(graw[:], gstage[:], rv4s[:],
                                            ALU.mult)
                    nc.scalar.activation(gnat[:], graw[:], AF.Exp)
                    nc.vector.tensor_scalar(gnat[:], gnat[:], 1.0, None,
                                            op0=ALU.add)
                    nc.vector.reciprocal(gnat3[:], gnat[:])
                    nc.vector.tensor_scalar(gnat3[:], gnat3[:], 32.0, None,
                                            op0=ALU.mult)
                if jf is not None:
                    nc.vector.reciprocal(rs[:], pot[:, :, 64:512:128])
                    nc.vector.tensor_tensor(crb[:], rs[:], gnat3[:, :, jf],
                                            ALU.mult)
                for skt in range(ST):
                    emit_scores_exp(j, skt, 0)
                    emit_scores_exp(j, skt, 1)
                    if jf is not None:
                        emit_fin(jf, skt, False)
                    if jc == 0:
                        emit_E(skt)
                    sched = avsched0 if jc == 0 else avsched
                    for sktp, q in sched.get(skt, []):
                        emit_attnv(j, sktp, q)

            # head 3, pass A: h=0 exps cover attnV for sq-tiles 0-7;
            # head-2 finalize interleaves as usual
            nc.vector.reciprocal(rs[:], pot[:, :, 64:512:128])
            nc.vector.tensor_tensor(crb[:], rs[:], gnat3[:, :, 2],
                                    ALU.mult)
            for skt in range(ST):
                emit_scores_exp(3, skt, 0)
                emit_fin(2, skt, False)
                for sktp, q in avsched.get(skt, []):
                    if q < 8:
                        emit_attnv(3, sktp, q)
            # pass B: h=1 exps; head-3's low sq-tiles finalize and the first
            # half of o_proj runs underneath them
            nc.vector.reciprocal(rs[:, 0:8], pot[:, 0:2, 64:512:128])
            nc.vector.tensor_tensor(crb[:, 0:8], rs[:, 0:8],
                                    gnat3[:, 0:8, 3], ALU.mult)
            for idx in range(ST):
                emit_scores_exp(3, idx, 1)
                if idx % 2 == 0:
                    emit_fin(3, idx // 2, False)
                else:
                    for q in range(8, 16):
                        emit_attnv(3, (idx - 1) // 2, q)
                if idx >= 7 and idx % 2 == 1:
                    emit_oproj((idx - 7) // 2)
            # pass C: high sq-tiles of head 3 + the rest of o_proj
            nc.vector.reciprocal(rs[:, 8:16], pot[:, 2:4, 64:512:128])
            nc.vector.tensor_tensor(crb[:, 8:16], rs[:, 8:16],
                                    gnat3[:, 8:16, 3], ALU.mult)
            oq = [4], [5], [6], [7, 8], [9], [10], [11], [12, 13, 14, 15]
            for i, q in enumerate(range(8, 16)):
                emit_fin(3, q, True)
                for t in oq[i]:
                    emit_oproj(t)
    return nc


def _get_program():
    global _NC_CACHE
    if _NC_CACHE is None:
        _NC_CACHE = _build_program()
    return _NC_CACHE


# ----------------------------------------------------------------------------
# host wrapper
# ----------------------------------------------------------------------------

def _prep_inputs(x, prenorm_w, qkv_w, gate_w, o_w, q_norm_w, k_norm_w):
    x = np.asarray(x, np.float32)
    pw = np.asarray(prenorm_w, np.float32)
    qkv_w = np.asarray(qkv_w, np.float32)
    gate_w = np.asarray(gate_w, np.float32)
    o_w = np.asarray(o_w, np.float32)
    qw = qkv_w[0:D] * pw[None, :]
    kw = qkv_w[D:2 * D] * pw[None, :]
    vw = qkv_w[2 * D:3 * D] * pw[None, :]
    gw = gate_w * pw[None, :]

    r = 1.0 / np.sqrt(np.mean(x * x, axis=-1) + EPS)      # [B, S]
    ind = np.zeros((P, 2), BF)
    ind[0:64, 0] = 1
    ind[64:128, 1] = 1
    ind2 = np.zeros((2, P), BF)
    ind2[0, 0:64] = 1
    ind2[1, 64:128] = 1
    qn = (np.tile(np.asarray(q_norm_w, np.float32), 2) / 32.0)[:, None]
    kn = (np.tile(np.asarray(k_norm_w, np.float32), 2) / 32.0)[:, None]

    in_maps = []
    for c in range(8):
        b, hg = c // 4, c % 4
        hsl = slice(256 * hg, 256 * hg + 256)
        xtc = np.ascontiguousarray(
            x[b].T.reshape(KTP, 2, P, S).transpose(2, 0, 1, 3)).astype(F8NP)
        wqk = np.concatenate([qw[hsl], kw[hsl]], 0).T * 32.0  # [1024, 512]
        wqkc = np.ascontiguousarray(
            wqk.reshape(KTP, 2, P, 512).transpose(2, 0, 1, 3)).astype(F8NP)
        wvg = np.concatenate([vw[hsl], gw[4 * hg:4 * hg + 4]], 0).T * 32.0
        wvgp = np.zeros((D, 272), np.float32)
        wvgp[:, 0:260] = wvg
        wvgc = np.ascontiguousarray(
            wvgp.reshape(KTP, 2, P, 272).transpose(2, 0, 1, 3)).astype(F8NP)
        wo = o_w[:, hsl].T.reshape(NH, 64, D).transpose(1, 0, 2) * 32.0
        woc = np.ascontiguousarray(wo).astype(F8NP)
        rvc = np.ascontiguousarray(
            r[b].reshape(ST, P).T / 32.0).astype(np.float32)
        rv4c = np.ascontiguousarray(np.repeat(-rvc, 4, axis=1))
        in_maps.append({
            "xt8": xtc, "wqk8": wqkc, "wvg8": wvgc, "wo8": woc,
            "rv": rvc, "rv4": rv4c, "qn": qn.astype(np.float32),
            "kn": kn.astype(np.float32), "ind": ind, "ind2": ind2,
        })
    return in_maps


_RUNNER = None


def _get_runner():
    """Build the sharded PJRT executable ONCE and reuse it across calls
    (run_bass_kernel_spmd re-traces/re-compiles on every invocation)."""
    global _RUNNER
    if _RUNNER is not None:
        return _RUNNER
    import jax
    import concourse.mybir as _mybir
    from concourse.bass2jax import (_bass_exec_p, partition_id_tensor,
                                    install_neuronx_cc_hook, Mesh,
                                    PartitionSpec, shard_map)
    install_neuronx_cc_hook()
    nc = _get_program()
    n_cores = 8
    partition_name = (nc.partition_id_tensor.name
                      if nc.partition_id_tensor else None)
    in_names, out_names, out_avals, zero_outs = [], [], [], []
    for alloc in nc.m.functions[0].allocations:
        if not isinstance(alloc, _mybir.MemoryLocationSet):
            continue
        name = alloc.memorylocations[0].name
        if alloc.kind == "ExternalInput":
            if name != partition_name:
                in_names.append(name)
        elif alloc.kind == "ExternalOutput":
            shape = tuple(alloc.tensor_shape)
            dtype = _mybir.dt.np(alloc.dtype)
            out_names.append(name)
            out_avals.append(jax.core.ShapedArray(shape, dtype))
            zero_outs.append(np.zeros(shape, dtype))
    n_params = len(in_names)
    n_outs = len(out_avals)
    all_in = list(in_names) + list(out_names)
    if partition_name is not None:
        all_in.append(partition_name)
    donate = tuple(range(n_params, n_params + n_outs))

    def _body(*args):
        operands = list(args)
        if partition_name is not None:
            operands.append(partition_id_tensor())
        return tuple(_bass_exec_p.bind(
            *operands, out_avals=tuple(out_avals), in_names=tuple(all_in),
            out_names=tuple(out_names), lowering_input_output_aliases=(),
            sim_require_finite=True, sim_require_nnan=True, nc=nc))

    devices = jax.devices()[:n_cores]
    mesh = Mesh(np.asarray(devices), ("core",))
    sharded = jax.jit(
        shard_map(_body, mesh=mesh,
                  in_specs=(PartitionSpec("core"),) * (n_params + n_outs),
                  out_specs=(PartitionSpec("core"),) * n_outs,
                  check_rep=False),
        donate_argnums=donate, keep_unused=True)
    _RUNNER = (sharded, in_names, out_names, out_avals, zero_outs, n_cores)
    return _RUNNER


def kernel(x, prenorm_w, qkv_w, gate_w, o_w, q_norm_w, k_norm_w):
    sharded, in_names, out_names, out_avals, zero_outs, n_cores = _get_runner()
    in_maps = _prep_inputs(x, prenorm_w, qkv_w, gate_w, o_w,
                           q_norm_w, k_norm_w)
    concat_in = [np.concatenate([in_maps[c][nm] for c in range(n_cores)], 0)
                 for nm in in_names]
    concat_zeros = [np.zeros((n_cores * z.shape[0], *z.shape[1:]), z.dtype)
                    for z in zero_outs]
    out_arrs = sharded(*concat_in, *concat_zeros)
    oi = out_names.index("out_p")
    op = np.asarray(out_arrs[oi]).astype(np.float32).reshape(
        n_cores, *out_avals[oi].shape)
    outs = [op[c] for c in range(n_cores)]
    x = np.asarray(x, np.float32)
    y0 = x[0] + outs[0] + outs[1] + outs[2] + outs[3]
    y1 = x[1] + outs[4] + outs[5] + outs[6] + outs[7]
    return np.stack([y0, y1]).astype(np.float32)
